# revision 1
# baseline (speedup 1.0000x reference)
"""Bass/Tile TRN2 kernel for nn_CenterAlignedTripletLoss (8-core SPMD).

Sharding (feature-parallel):
  Phase 1: feats sharded along the feature axis (8192 -> 1024 per core).
    Each core computes partial mining scores for all 96 centers over its
    slice:  s[n, m] = ||f_m||^2 - 2 c_n . f_m - BIGM*(1-is_pos[n,m])
    accumulated in PSUM (the label mask enters as a rank-33 matmul
    k-tile; partial norms as a rank-1 f32 k-tile; the row-constant
    ||c_n||^2 is omitted).
  Exchange (Mesh AllToAll x2): bf16 center slices (so each core gets
    full-feature centers for its 12-center block), then f32 partial
    scores; a fold matmul sums the 8 receive blocks into full masked
    scores [12, 1536].
  Mining: hardest positive = row argmax, hardest negative = row argmax
    of the negated matrix (one masked matrix serves both).
  Phase 2: per-stripe indirect-DMA gathers of the winning
    local_features rows (bf16, direct to (center,stripe) partition
    layout), stripe distance matrices via DVE diff + ACT
    square-accumulate (positive set) and DVE mult + GpSimd avg-pool
    (negative set), sqrt -> tanh(d/2), anti-diagonal shortest-path DP,
    partial sum of relu(ap - an + margin).  Host sums 8 partials.
"""

import numpy as np
from contextlib import ExitStack

import concourse.bass as bass
import concourse.bacc as bacc
import concourse.tile as tile
from concourse import mybir
from concourse import bass_utils

F32 = mybir.dt.float32
BF16 = mybir.dt.bfloat16
U32 = mybir.dt.uint32
AF = mybir.ActivationFunctionType
ALU = mybir.AluOpType
PF = mybir.PoolFunctionType

NCORES = 8
M = 1536          # samples
D = 8192          # feature dim
DPC = D // NCORES # 1024 features per core
N = 96            # centers
NB = N // NCORES  # 12 centers per core
K = 16            # samples per chunk
S = 8             # stripes
DL = 1024         # local feature dim per stripe
L = 32            # label count
MARGIN = 0.3
BIGM = 16384.0    # per-core mask magnitude (2^14; sums to 2^17)
BIGDP = 1.0e6     # DP pad value
RG = [list(range(NCORES))]
NT = M // 128     # 12 sample tiles


def build_body(tc, out, ins):
    nc = tc.nc
    fnat = ins["fnat"]          # [M, DPC] natural feature slice (bf16)
    ftT = ins["ftT"]            # [DPC, M] pre-transposed slice (bf16)
    lf = ins["lf"]              # [M, D] local_features (stripe, d) bf16
    um = ins["um"]              # [33, 96] mask lhsT (bf16)
    vm = ins["vm"]              # [33, M] mask rhs (bf16)
    fsel = ins["fsel"]          # [96, NB] fold selector (f32)
    wsel = ins["wsel"]          # [64, NB] +I / -I rows (f32)
    ident = ins["ident"]        # [128, 128] identity (f32)
    esel = ins["esel"]          # [96, 96] grid-fold selection (bf16)
    repsel = ins["repsel"]      # [NB, 96] replicate selector (bf16)
    aavg = ins["aavg"]          # [128, NT*N] packed chunk-avg matrix bf16

    lf_v = lf.rearrange("m (j d) -> (m j) d", d=DL)  # [12288, 1024] stripe rows

    with ExitStack() as ctx:
        const = ctx.enter_context(tc.tile_pool(name="const", bufs=1))
        pers = ctx.enter_context(tc.tile_pool(name="pers", bufs=1))
        dram = ctx.enter_context(tc.tile_pool(name="dram", bufs=1, space="DRAM"))

        # DRAM bounce buffers for collectives
        a2a_in = dram.tile([N, M], F32)
        a2a_out = dram.tile([N, M], F32)
        c2a_in = dram.tile([N, DPC], BF16)
        c2a_out = dram.tile([N, DPC], BF16)

        # ---- big loads first so queues fill immediately ----
        nt_sb = [const.tile([128, DPC], BF16, name=f"nt{t}") for t in range(NT)]
        for t in range(NT):
            nc.sync.dma_start(nt_sb[t], fnat[t * 128:(t + 1) * 128, :])
        ftT_sb = [const.tile([128, M], BF16, name=f"ftT{kd}") for kd in range(8)]
        for kd in range(8):
            nc.sync.dma_start(ftT_sb[kd], ftT[kd * 128:(kd + 1) * 128, :])
        ident_sb = const.tile([128, 128], F32)
        nc.sync.dma_start(ident_sb, ident)
        wsel_sb = const.tile([64, NB], F32)
        nc.sync.dma_start(wsel_sb, wsel)
        esel_sb = const.tile([N, N], BF16)
        nc.sync.dma_start(esel_sb, esel)
        um_sb = const.tile([33, N], BF16)
        nc.sync.dma_start(um_sb, um)
        vm_sb = const.tile([33, M], BF16)
        nc.sync.dma_start(vm_sb, vm)
        fsel_sb = const.tile([N, NB], F32)
        nc.sync.dma_start(fsel_sb, fsel)
        rep_sb = const.tile([NB, N], BF16)
        nc.sync.dma_start(rep_sb, repsel)
        a_sb = const.tile([128, NT * N], BF16)
        nc.sync.dma_start(a_sb, aavg)
        ones_c = const.tile([NB, 1], F32)
        nc.vector.memset(ones_c, 1.0)
        marg = const.tile([NB, 1], F32)
        nc.vector.memset(marg, MARGIN)
        idx0 = const.tile([2, 1], U32)
        nc.vector.memset(idx0, 0)

        ctr_cm = pers.tile([N, DPC], F32)   # center-major centers slice (f32)
        ctr_bf = pers.tile([N, DPC], BF16)  # bf16 copy for the exchange
        x_all = pers.tile([N, D], BF16)     # centers replicated over stripes
        nrow = pers.tile([1, M], F32)       # partial sample norms row
        normcol = pers.tile([128, NT], F32)
        s_sb = pers.tile([N, M], F32)

        # phase-2 grids (memsets early; off the critical path)
        dD = pers.tile([64, 81], F32)
        nc.vector.memset(dD, 0.0)
        dp = pers.tile([64, 81], F32)
        nc.vector.memset(dp, BIGDP)
        nc.vector.memset(dp[:, 1:2], 0.0)

        # ---- phase 1 ----
        with tc.tile_pool(name="psumc", bufs=1, space="PSUM") as psumc, \
             tc.tile_pool(name="psums", bufs=1, space="PSUM") as psums, \
             tc.tile_pool(name="psumt", bufs=2, space="PSUM") as psumt, \
             tc.tile_pool(name="workp1", bufs=2) as work:
            ps_c = [psumc.tile([N, 512], F32, name=f"ps_c{h}") for h in range(2)]
            s_ps = [psums.tile([N, 512], F32, name=f"s_ps{b}") for b in range(3)]
            for t in range(NT):
                for h in range(2):
                    nc.tensor.matmul(
                        ps_c[h], lhsT=a_sb[:, t * N:(t + 1) * N],
                        rhs=nt_sb[t][:, h * 512:(h + 1) * 512],
                        start=(t == 0), stop=(t == NT - 1),
                    )
                sqd = work.tile([128, DPC], F32, tag="sqd")
                nc.scalar.activation(
                    sqd, nt_sb[t], AF.Square, accum_out=normcol[:, t:t + 1]
                )
            # centers psum -> f32 (DVE, for transposes) + bf16 (ACT, exchange)
            for h in range(2):
                nc.vector.tensor_copy(ctr_cm[:, h * 512:(h + 1) * 512], ps_c[h])
                nc.scalar.activation(
                    ctr_bf[:, h * 512:(h + 1) * 512], ps_c[h], AF.Copy
                )
            nc.sync.dma_start(c2a_in, ctr_bf)
            nc.gpsimd.collective_compute(
                "AllToAll", ALU.bypass, replica_groups=RG,
                ins=[c2a_in.opt()], outs=[c2a_out.opt()],
            )

            # mask k-tile first (only needs consts)
            for b in range(3):
                nc.tensor.matmul(
                    s_ps[b], lhsT=um_sb, rhs=vm_sb[:, b * 512:(b + 1) * 512],
                    start=True, stop=False,
                )
            # ctrT blocks scaled by -2 for the dot matmuls
            ctrT2 = []
            for kd in range(8):
                tpc = psumt.tile([128, N], F32, tag="tp")
                nc.tensor.transpose(
                    tpc, ctr_cm[:, kd * 128:(kd + 1) * 128], ident_sb[:N, :N]
                )
                c2 = work.tile([128, N], BF16, tag=f"ctrT{kd}", bufs=1)
                nc.vector.tensor_scalar_mul(c2, tpc, -2.0)
                ctrT2.append(c2)
            # norm row [1, 1536] from normcol via transposes
            ones_n = const.tile([1, N], F32)
            nc.vector.memset(ones_n, 1.0)
            for t in range(NT):
                tpn = psumt.tile([1, 128], F32, tag="tpn", bufs=1)
                nc.tensor.transpose(tpn, normcol[:, t:t + 1], ident_sb)
                nc.vector.tensor_copy(nrow[:, t * 128:(t + 1) * 128], tpn)
            # scores: accumulate -2 c.f over 8 k-tiles, then the norm row
            for kd in range(8):
                for b in range(3):
                    nc.tensor.matmul(
                        s_ps[b], lhsT=ctrT2[kd],
                        rhs=ftT_sb[kd][:, b * 512:(b + 1) * 512],
                        start=False, stop=False,
                    )
            for b in range(3):
                nc.tensor.matmul(
                    s_ps[b], lhsT=ones_n, rhs=nrow[:, b * 512:(b + 1) * 512],
                    start=False, stop=True,
                )
            for b in range(3):
                nc.vector.tensor_copy(s_sb[:, b * 512:(b + 1) * 512], s_ps[b])
            nc.sync.dma_start(a2a_in, s_sb)
            # dummy indirect gather: absorbs the SWDGE drain before the
            # real gathers land on the critical path
            dscr = pers.tile([2, 64], BF16)
            nc.gpsimd.indirect_dma_start(
                out=dscr, out_offset=None, in_=lf_v[:, :64],
                in_offset=bass.IndirectOffsetOnAxis(ap=idx0, axis=0),
            )
            nc.gpsimd.collective_compute(
                "AllToAll", ALU.bypass, replica_groups=RG,
                ins=[a2a_in.opt()], outs=[a2a_out.opt()],
            )

        # ---- x_all: replicate centers over stripe partitions via PE ----
        # (runs while the scores AllToAll is on the wire)
        xc3 = pers.tile([NB, S, DL], BF16)
        nc.sync.dma_start(xc3, c2a_out.rearrange("(i b) d -> b i d", b=NB))
        with tc.tile_pool(name="psumx", bufs=4, space="PSUM") as psumx:
            for i in range(S):
                for h in range(2):
                    xr = psumx.tile([N, 512], F32, tag="xr")
                    nc.tensor.matmul(
                        xr, lhsT=rep_sb,
                        rhs=xc3[:, i, h * 512:(h + 1) * 512],
                        start=True, stop=True,
                    )
                    lo = i * DL + h * 512
                    if i % 2 == 0:
                        nc.vector.tensor_copy(x_all[:, lo:lo + 512], xr)
                    else:
                        nc.scalar.activation(x_all[:, lo:lo + 512], xr, AF.Copy)

        # ---- fold + mining on this core's 12 rows ----
        ctx2 = tc.tile_pool(name="mine", bufs=1)
        mine = ctx2.__enter__()
        pmx = tc.tile_pool(name="psumm", bufs=1, space="PSUM")
        psumm = pmx.__enter__()
        s8 = mine.tile([N, M], F32, name="s8")
        nc.sync.dma_start(s8, a2a_out)
        pm = psumm.tile([NB, M], F32)
        for b in range(3):
            nc.tensor.matmul(
                pm[:, b * 512:(b + 1) * 512], lhsT=fsel_sb,
                rhs=s8[:, b * 512:(b + 1) * 512], start=True, stop=True,
            )
        pmax = pers.tile([NB, 8], F32)
        pidx = pers.tile([NB, 8], U32)
        nmax = pers.tile([NB, 8], F32)
        nidx = pers.tile([NB, 8], U32)
        nneg = mine.tile([NB, M], F32, name="nneg")
        nc.vector.max(pmax, pm)
        nc.vector.max_index(pidx, pmax, pm)
        nc.vector.tensor_scalar_mul(nneg, pm, -1.0)
        nc.vector.max(nmax, nneg)
        nc.vector.max_index(nidx, nmax, nneg)

        # ---- phase 2: per-stripe gathers + local distances ----
        y96p = pers.tile([N, DL], BF16)
        y96n = pers.tile([N, DL], BF16)
        y3p = y96p.rearrange("(b j) d -> b j d", j=S)
        y3n = y96n.rearrange("(b j) d -> b j d", j=S)
        for iset, (idx, y3) in enumerate(((pidx, y3p), (nidx, y3n))):
            for j in range(S):
                pj = pers.tile([NB, 1], U32, tag="pj", bufs=4)
                nc.vector.tensor_scalar(
                    out=pj, in0=idx[:, :1], scalar1=8, scalar2=j,
                    op0=ALU.mult, op1=ALU.add,
                )
                nc.gpsimd.indirect_dma_start(
                    out=y3[:, j, :], out_offset=None, in_=lf_v,
                    in_offset=bass.IndirectOffsetOnAxis(ap=pj, axis=0),
                )
        pmx.__exit__(None, None, None)
        ctx2.__exit__(None, None, None)

        d2t = pers.tile([N, 2 * S], F32)   # cols 0-7 pos, 8-15 neg
        mprod = pers.tile([N, S, DL], BF16)
        with tc.tile_pool(name="workp2", bufs=2) as work2:
            for i in range(S):
                dtp = work2.tile([N, DL], BF16, tag="dtp")
                nc.vector.tensor_tensor(
                    out=dtp, in0=y96p, in1=x_all[:, i * DL:(i + 1) * DL],
                    op=ALU.subtract,
                )
                sqp = work2.tile([N, DL], BF16, tag="sqp")
                nc.scalar.activation(
                    sqp, dtp, AF.Square, accum_out=d2t[:, i:i + 1]
                )
                dtn = work2.tile([N, DL], BF16, tag="dtn")
                nc.vector.tensor_tensor(
                    out=dtn, in0=y96n, in1=x_all[:, i * DL:(i + 1) * DL],
                    op=ALU.subtract,
                )
                nc.vector.tensor_tensor(
                    out=mprod[:, i, :], in0=dtn, in1=dtn, op=ALU.mult
                )
            nc.vector.tensor_reduce(
                d2t[:, S:2 * S], mprod, axis=mybir.AxisListType.X, op=ALU.add
            )

            dcl = work2.tile([N, 2 * S], F32, tag="dcl", bufs=1)
            nc.vector.tensor_scalar_max(dcl, d2t, 1e-12)
            dsqr = work2.tile([N, 2 * S], F32, tag="dsqr", bufs=1)
            nc.scalar.activation(dsqr, dcl, AF.Sqrt)
            dsg = work2.tile([N, 2 * S], BF16, tag="dsg", bufs=1)
            nc.scalar.activation(dsg, dsqr, AF.Tanh, scale=0.5)
            # fold [96 (b,j), 8 (i)] -> dD[b + iset*32, 9*i + j + 1] via PE
            for iset in range(2):
                with tc.tile_pool(name=f"psf{iset}", bufs=1, space="PSUM") as psf_p:
                    psf = psf_p.tile([NB, 64], F32)
                    for j in range(S):
                        nc.tensor.matmul(
                            psf[:, j * S:(j + 1) * S],
                            lhsT=esel_sb[:, j * NB:(j + 1) * NB],
                            rhs=dsg[:, iset * S:(iset + 1) * S],
                            start=True, stop=True,
                        )
                    dst = dD[iset * 32:iset * 32 + NB, 0:72].rearrange(
                        "p (i j) -> p j i", j=9
                    )[:, 1:9, :]
                    nc.vector.tensor_copy(
                        dst, psf.rearrange("p (j i) -> p j i", i=S)
                    )

        # ---- shortest-path DP on anti-diagonals (9x9 padded grid) ----
        for kdiag in range(2, 17):
            lo = max(1, kdiag - 8)
            hi = min(8, kdiag - 1)
            cnt = hi - lo + 1
            f0 = 9 * lo + (kdiag - lo)
            t = pers.tile([64, 8], F32, tag="dptmp", bufs=2)
            nc.vector.tensor_tensor(
                out=t[:, :cnt],
                in0=dp[:, f0 - 9:f0 - 9 + 8 * (cnt - 1) + 1:8],
                in1=dp[:, f0 - 1:f0 - 1 + 8 * (cnt - 1) + 1:8],
                op=ALU.min,
            )
            nc.vector.tensor_tensor(
                out=dp[:, f0:f0 + 8 * (cnt - 1) + 1:8],
                in0=t[:, :cnt],
                in1=dD[:, f0 - 9:f0 - 9 + 8 * (cnt - 1) + 1:8],
                op=ALU.add,
            )

        # ---- loss: relu(ap - an + margin), partial sum over 12 centers ----
        with tc.tile_pool(name="psum3", bufs=1, space="PSUM") as psum3:
            dps = psum3.tile([NB, 1], F32)
            nc.tensor.matmul(dps, lhsT=wsel_sb, rhs=dp[:, 80:81], start=True, stop=True)
            r12 = pers.tile([NB, 1], F32)
            nc.vector.tensor_scalar(
                out=r12, in0=dps, scalar1=marg, scalar2=0.0,
                op0=ALU.add, op1=ALU.max,
            )
            lsum = psum3.tile([1, 1], F32)
            nc.tensor.matmul(lsum, lhsT=r12, rhs=ones_c, start=True, stop=True)
            out_sb = pers.tile([1, 1], F32)
            nc.vector.tensor_copy(out_sb, lsum)
        nc.sync.dma_start(out, out_sb)


def build_program():
    nc = bacc.Bacc(
        "TRN2", target_bir_lowering=False, debug=False,
        enable_asserts=False, num_devices=NCORES,
    )
    ins = {
        "fnat": nc.dram_tensor("fnat", [M, DPC], BF16, kind="ExternalInput").ap(),
        "ftT": nc.dram_tensor("ftT", [DPC, M], BF16, kind="ExternalInput").ap(),
        "lf": nc.dram_tensor("lf", [M, D], BF16, kind="ExternalInput").ap(),
        "um": nc.dram_tensor("um", [33, N], BF16, kind="ExternalInput").ap(),
        "vm": nc.dram_tensor("vm", [33, M], BF16, kind="ExternalInput").ap(),
        "fsel": nc.dram_tensor("fsel", [N, NB], F32, kind="ExternalInput").ap(),
        "wsel": nc.dram_tensor("wsel", [64, NB], F32, kind="ExternalInput").ap(),
        "ident": nc.dram_tensor("ident", [128, 128], F32, kind="ExternalInput").ap(),
        "esel": nc.dram_tensor("esel", [N, N], BF16, kind="ExternalInput").ap(),
        "repsel": nc.dram_tensor("repsel", [NB, N], BF16, kind="ExternalInput").ap(),
        "aavg": nc.dram_tensor("aavg", [128, NT * N], BF16, kind="ExternalInput").ap(),
    }
    out = nc.dram_tensor("out", [1, 1], F32, kind="ExternalOutput").ap()
    with tile.TileContext(nc) as tc:
        build_body(tc, out, ins)
    nc.compile()
    return nc


def make_in_maps(feats, labels, local_features):
    bf16 = mybir.dt.np(BF16)
    feats = np.asarray(feats, dtype=np.float32).astype(bf16)
    labels = np.asarray(labels).astype(np.int64)
    lf_flat = np.ascontiguousarray(
        np.asarray(local_features, dtype=np.float32)
        .transpose(0, 2, 1).reshape(M, D)
    ).astype(bf16)
    anchors = labels[::K]  # [96]
    # mask factors: sum_r um[r,n]*vm[r,m] = -BIGM * (anchors[n] != labels[m])
    um = np.zeros((33, N), dtype=np.float32)
    vm = np.zeros((33, M), dtype=np.float32)
    um[0, :] = -BIGM
    vm[0, :] = 1.0
    for ell in range(L):
        um[1 + ell, :] = BIGM * (anchors == ell)
        vm[1 + ell, :] = (labels == ell)
    um = um.astype(bf16)
    vm = vm.astype(bf16)
    # fold selector: a2a_out row (NB*i + b) -> out row b
    fsel = np.zeros((N, NB), dtype=np.float32)
    for i in range(NCORES):
        for b in range(NB):
            fsel[NB * i + b, b] = 1.0
    wsel = np.zeros((64, NB), dtype=np.float32)
    wsel[0:NB, :] = np.eye(NB)
    wsel[32:32 + NB, :] = -np.eye(NB)
    ident = np.eye(128, dtype=np.float32)
    esel = np.zeros((N, N), dtype=np.float32)
    for b in range(NB):
        for j in range(S):
            esel[b * S + j, j * NB + b] = 1.0
    esel = esel.astype(bf16)
    # replicate selector: center b -> partitions b*8..b*8+8
    repsel = np.zeros((NB, N), dtype=np.float32)
    for b in range(NB):
        repsel[b, b * S:(b + 1) * S] = 1.0
    repsel = repsel.astype(bf16)
    # packed chunk-averaging matrix: [128, t*96+n]
    aavg = np.zeros((M, N), dtype=np.float32)
    aavg[np.arange(M), np.arange(M) // K] = 1.0 / K
    aavg_p = np.ascontiguousarray(
        aavg.reshape(NT, 128, N).transpose(1, 0, 2).reshape(128, NT * N)
    ).astype(bf16)
    in_maps = []
    for c in range(NCORES):
        sl = feats[:, c * DPC:(c + 1) * DPC]
        in_maps.append({
            "fnat": np.ascontiguousarray(sl),
            "ftT": np.ascontiguousarray(sl.T),
            "lf": lf_flat,
            "um": um,
            "vm": vm,
            "fsel": fsel,
            "wsel": wsel,
            "ident": ident,
            "esel": esel,
            "repsel": repsel,
            "aavg": aavg_p,
        })
    return in_maps


_NC_CACHE = None


def _get_nc():
    global _NC_CACHE
    if _NC_CACHE is None:
        _NC_CACHE = build_program()
    return _NC_CACHE


def run(feats, labels, local_features, trace=False, **kwargs):
    nc = _get_nc()
    in_maps = make_in_maps(feats, labels, local_features)
    res = bass_utils.run_bass_kernel_spmd(
        nc, in_maps, core_ids=list(range(NCORES)), trace=trace, **kwargs
    )
    partial = sum(float(r["out"][0, 0]) for r in res.results)
    return np.float32(partial / N), res


def kernel(feats, labels, local_features):
    loss, _ = run(feats, labels, local_features)
    return loss



# revision 17
# speedup vs baseline: 1.2549x; 1.2549x over previous
"""Bass/Tile TRN2 kernel for nn_CenterAlignedTripletLoss (8-core SPMD).

Sharding (feature-parallel):
  Phase 1: feats sharded along the feature axis (8192 -> 1024 per core).
    Each core computes partial mining scores for all 96 centers over its
    slice:  s[n, m] = ||f_m||^2 - 2 c_n . f_m - BIGM*(1-is_pos[n,m])
    accumulated in PSUM (the label mask enters as a rank-33 matmul
    k-tile; partial norms as a rank-1 f32 k-tile; the row-constant
    ||c_n||^2 is omitted).  Norm tiles split ACT/DVE (tensor_tensor_
    reduce).  A tiny warmup AllToAll issued first absorbs the initial
    cross-core barrier + CC setup into the load phase.
  Exchange: Mesh AllToAll of bf16 center slices (each core gets
    full-feature centers for its 12-center block), then ReduceScatter
    (add) of the f32 partial scores so each core receives its 12 rows
    already summed -- no fold matmul.
  Mining: hardest positive = row argmax, hardest negative = row argmax
    of the negated matrix.
  Phase 2: one indirect-DMA gather per set (12 descriptors x 16KB: the
    winner's full 8-stripe local row, landing as (center,stripe)
    partition layout), stripe distances via GpSimd subtract + ACT
    square-accumulate (positive) / DVE tensor_tensor_reduce (negative),
    sqrt -> tanh(d/2) (sqrt table preloaded off-path), PE fold to the
    DP grid, then the AlignedReID shortest path as 8 row-wise
    tensor_tensor_scan(min, add) ops.  Partial loss summed on host.
"""

import os
import numpy as np
from contextlib import ExitStack

import concourse.bass as bass
import concourse.bacc as bacc
import concourse.tile as tile
from concourse import mybir
from concourse import bass_utils

F32 = mybir.dt.float32
BF16 = mybir.dt.bfloat16
U32 = mybir.dt.uint32
AF = mybir.ActivationFunctionType
ALU = mybir.AluOpType

NCORES = 8
M = 1536          # samples
D = 8192          # feature dim
DPC = D // NCORES # 1024 features per core
N = 96            # centers
NB = N // NCORES  # 12 centers per core
K = 16            # samples per chunk
S = 8             # stripes
DL = 1024         # local feature dim per stripe
L = 32            # label count
MARGIN = 0.3
BIGM = 16384.0    # per-core mask magnitude (sums to 2^17 after RS)
BIGDP = 1.0e6     # DP "no predecessor" value
RG = [list(range(NCORES))]
NT = M // 128     # 12 sample tiles

DEBUG = False     # add intermediate ExternalOutputs (test harness only)


def _flag(name, default="1"):
    return os.environ.get(name, default) == "1"


F_WARMUP = _flag("K_WARMUP")       # tiny warmup collective first
F_RS = _flag("K_RS")               # ReduceScatter scores (else A2A + fold)
# gather mode: 0 = 16 per-stripe gathers, 1 = 16KB-elem (2 gathers),
# 2 = 96-row batched (2 gathers + index replicate matmul)
F_BIGGATHER = int(os.environ.get("K_BIGGATHER", "2"))
F_SCAN = _flag("K_SCAN")           # tensor_tensor_scan DP (else antidiagonal)
F_XDMA = _flag("K_XDMA")           # x_all replicate via DMA (else PE matmuls)
F_TTR = _flag("K_TTR")             # tensor_tensor_reduce for norms/neg dist
F_GPS = _flag("K_GPS")             # gpsimd for subtracts/negate


def build_body(tc, outs, ins):
    nc = tc.nc
    fnat = ins["fnat"]          # [M, DPC] natural feature slice (bf16)
    ftT = ins["ftT"]            # [DPC, M] pre-transposed slice (bf16)
    lf = ins["lf"]              # [M, D] local_features (stripe, d) bf16
    um = ins["um"]              # [33, 96] mask lhsT (bf16)
    vm = ins["vm"]              # [33, M] mask rhs (bf16)
    wsel = ins["wsel"]          # [64, NB] +I / -I rows (f32)
    ident = ins["ident"]        # [128, 128] identity (f32)
    esel = ins["esel"]          # [96, 96] grid-fold selection (bf16)
    aavg = ins["aavg"]          # [128, NT*N] packed chunk-avg matrix bf16
    fsel = ins["fsel"]          # [96, NB] fold selector (f32, fallback)
    repsel = ins["repsel"]      # [NB, 96] replicate selector (bf16, fallback)

    lf4 = lf.rearrange("m (j d) -> m j d", d=DL)   # [1536, 8, 1024]
    lf_v = lf.rearrange("m (j d) -> (m j) d", d=DL)

    sub_eng = nc.gpsimd if F_GPS else nc.vector

    with ExitStack() as ctx:
        const = ctx.enter_context(tc.tile_pool(name="const", bufs=1))
        pers = ctx.enter_context(tc.tile_pool(name="pers", bufs=1))
        dram = ctx.enter_context(tc.tile_pool(name="dram", bufs=1, space="DRAM"))

        # DRAM bounce buffers for collectives
        warm_in = dram.tile([NCORES, 1], F32)
        warm_out = dram.tile([NCORES, 1], F32)
        c2a_in = dram.tile([N, DPC], BF16)
        c2a_out = dram.tile([N, DPC], BF16)
        rs_in = dram.tile([N, M], F32)
        rs_out = dram.tile([NB if F_RS else N, M], F32)
        nrow_d = dram.tile([NT, 128], F32)

        # ---- warmup collective first: absorbs the initial cross-core
        # barrier + CC ring setup while the loads/compute run ----
        if F_WARMUP:
            nc.gpsimd.collective_compute(
                "AllToAll", ALU.bypass, replica_groups=RG,
                ins=[warm_in.opt()], outs=[warm_out.opt()],
            )

        # ---- big loads first so queues fill immediately ----
        nt_sb = [const.tile([128, DPC], BF16, name=f"nt{t}") for t in range(NT)]
        for t in range(NT):
            nc.sync.dma_start(nt_sb[t], fnat[t * 128:(t + 1) * 128, :])
        ftT_sb = [const.tile([128, M], BF16, name=f"ftT{kd}") for kd in range(8)]
        for kd in range(8):
            nc.sync.dma_start(ftT_sb[kd], ftT[kd * 128:(kd + 1) * 128, :])
        ident_sb = const.tile([128, 128], F32)
        nc.sync.dma_start(ident_sb, ident)
        wsel_sb = const.tile([64, NB], F32)
        nc.sync.dma_start(wsel_sb, wsel)
        esel_sb = const.tile([N, N], BF16)
        nc.sync.dma_start(esel_sb, esel)
        um_sb = const.tile([33, N], BF16)
        nc.sync.dma_start(um_sb, um)
        vm_sb = const.tile([33, M], BF16)
        nc.sync.dma_start(vm_sb, vm)
        if not F_RS:
            fsel_sb = const.tile([N, NB], F32)
            nc.sync.dma_start(fsel_sb, fsel)
        if not F_XDMA:
            rep_sb = const.tile([NB, N], BF16)
            nc.sync.dma_start(rep_sb, repsel)
        if F_BIGGATHER == 2:
            rep8_sb = const.tile([NB, N], F32)
            nc.sync.dma_start(rep8_sb, ins["rep8"])
            joff_sb = const.tile([N, 1], F32)
            nc.sync.dma_start(joff_sb, ins["joff"])
            ins["rep8_sb"] = rep8_sb
            ins["joff_sb"] = joff_sb
        ones_c = const.tile([NB, 1], F32)
        nc.vector.memset(ones_c, 1.0)
        marg = const.tile([NB, 1], F32)
        nc.vector.memset(marg, MARGIN)
        idx0 = const.tile([2, 1], U32)
        nc.vector.memset(idx0, 0)
        a_sb = const.tile([128, NT * N], BF16)
        nc.sync.dma_start(a_sb, aavg)

        ctr_cm = pers.tile([N, DPC], F32)   # center-major centers slice (f32)
        ctr_bf = pers.tile([N, DPC], BF16)  # bf16 copy for the exchange
        x_all = pers.tile([N, D], BF16)     # centers replicated over stripes
        nrow = pers.tile([1, M], F32)       # partial sample norms row
        normcol = pers.tile([128, NT], F32)
        normT_sb = pers.tile([NT, 128], F32)

        # phase-2 grids
        if F_SCAN:
            dgrid = pers.tile([64, 64], F32)    # [b(+32 for neg), 8i x 8j]
            nc.vector.memset(dgrid, 0.0)
            bigrow = pers.tile([64, S], F32)
            nc.vector.memset(bigrow, BIGDP)
            dp_sb = pers.tile([64, 64], F32)    # DP rows
        else:
            dD = pers.tile([64, 81], F32)
            nc.vector.memset(dD, 0.0)
            dp = pers.tile([64, 81], F32)
            nc.vector.memset(dp, BIGDP)
            nc.vector.memset(dp[:, 1:2], 0.0)

        # ---- phase 1 ----
        with tc.tile_pool(name="psumc", bufs=1, space="PSUM") as psumc, \
             tc.tile_pool(name="psums", bufs=1, space="PSUM") as psums, \
             tc.tile_pool(name="psumt", bufs=2, space="PSUM") as psumt, \
             tc.tile_pool(name="workp1", bufs=2) as work:
            ps_c = [psumc.tile([N, 512], F32, name=f"ps_c{h}") for h in range(2)]
            s_ps = [psums.tile([N, 512], F32, name=f"s_ps{b}") for b in range(3)]
            for t in range(NT):
                for h in range(2):
                    nc.tensor.matmul(
                        ps_c[h], lhsT=a_sb[:, t * N:(t + 1) * N],
                        rhs=nt_sb[t][:, h * 512:(h + 1) * 512],
                        start=(t == 0), stop=(t == NT - 1),
                    )
                # partial sample norms: split ACT / DVE to halve the chain
                if F_TTR and t % 2 == 1:
                    sqv = work.tile([128, DPC], BF16, tag="sqv")
                    nc.vector.tensor_tensor(
                        out=sqv, in0=nt_sb[t], in1=nt_sb[t], op=ALU.mult
                    )
                    nc.vector.tensor_reduce(
                        normcol[:, t:t + 1], sqv,
                        axis=mybir.AxisListType.X, op=ALU.add,
                    )
                else:
                    sqd = work.tile([128, DPC], F32, tag="sqd")
                    nc.scalar.activation(
                        sqd, nt_sb[t], AF.Square, accum_out=normcol[:, t:t + 1]
                    )
            # centers psum -> f32 (DVE, for transposes) + bf16 (ACT, exchange)
            for h in range(2):
                nc.vector.tensor_copy(ctr_cm[:, h * 512:(h + 1) * 512], ps_c[h])
                nc.scalar.activation(
                    ctr_bf[:, h * 512:(h + 1) * 512], ps_c[h], AF.Copy
                )
            nc.sync.dma_start(c2a_in, ctr_bf)
            nc.gpsimd.collective_compute(
                "AllToAll", ALU.bypass, replica_groups=RG,
                ins=[c2a_in.opt()], outs=[c2a_out.opt()],
            )

            # mask k-tile first (only needs consts)
            for b in range(3):
                nc.tensor.matmul(
                    s_ps[b], lhsT=um_sb, rhs=vm_sb[:, b * 512:(b + 1) * 512],
                    start=True, stop=False,
                )
            # ctrT blocks scaled by -2 for the dot matmuls
            ctrT2 = []
            for kd in range(8):
                tpc = psumt.tile([128, N], F32, tag="tp")
                nc.tensor.transpose(
                    tpc, ctr_cm[:, kd * 128:(kd + 1) * 128], ident_sb[:N, :N]
                )
                c2 = work.tile([128, N], BF16, tag=f"ctrT{kd}", bufs=1)
                nc.vector.tensor_scalar_mul(c2, tpc, -2.0)
                ctrT2.append(c2)
            # norm row [1, 1536]: one transpose + DRAM-bounced rearrange
            ones_n = const.tile([1, N], F32)
            nc.vector.memset(ones_n, 1.0)
            tpn = psumt.tile([NT, 128], F32, tag="tpn", bufs=1)
            nc.tensor.transpose(tpn, normcol, ident_sb)
            nc.vector.tensor_copy(normT_sb, tpn)
            nc.sync.dma_start(nrow_d, normT_sb)
            nc.sync.dma_start(nrow, nrow_d.flatten().unsqueeze(0))
            # scores: accumulate -2 c.f over 8 k-tiles, then the norm row
            for kd in range(8):
                for b in range(3):
                    nc.tensor.matmul(
                        s_ps[b], lhsT=ctrT2[kd],
                        rhs=ftT_sb[kd][:, b * 512:(b + 1) * 512],
                        start=False, stop=False,
                    )
            for b in range(3):
                nc.tensor.matmul(
                    s_ps[b], lhsT=ones_n, rhs=nrow[:, b * 512:(b + 1) * 512],
                    start=False, stop=True,
                )
            s_sb = pers.tile([N, M], F32)
            for b in range(3):
                nc.vector.tensor_copy(s_sb[:, b * 512:(b + 1) * 512], s_ps[b])
            nc.sync.dma_start(rs_in, s_sb)
            # dummy indirect gather: absorbs the SWDGE drain before the
            # real gathers land on the critical path
            dscr = pers.tile([2, 64], BF16)
            nc.gpsimd.indirect_dma_start(
                out=dscr, out_offset=None, in_=lf_v[:, :64],
                in_offset=bass.IndirectOffsetOnAxis(ap=idx0, axis=0),
            )
            if F_RS:
                nc.gpsimd.collective_compute(
                    "ReduceScatter", ALU.add, replica_groups=RG,
                    ins=[rs_in.opt()], outs=[rs_out.opt()],
                )
            else:
                nc.gpsimd.collective_compute(
                    "AllToAll", ALU.bypass, replica_groups=RG,
                    ins=[rs_in.opt()], outs=[rs_out.opt()],
                )

        # preload the Sqrt activation table (slot beside Square) while the
        # collectives are on the wire, so phase 2 pays no sqrt table load
        sq_warm = pers.tile([1, S], F32)
        nc.vector.memset(sq_warm, 1.0)
        sq_warm2 = pers.tile([1, S], F32)
        nc.scalar.activation(sq_warm2, sq_warm, AF.Sqrt)

        # ---- x_all: replicate centers over stripe partitions ----
        # (runs while the scores exchange is on the wire)
        if F_XDMA:
            c2a3 = c2a_out.rearrange("(i b) d -> b i d", b=NB)   # [12, 8, 1024]
            x4 = x_all.rearrange("(b j) (i d) -> b j i d", j=S, d=DL)
            for j in range(S):
                nc.sync.dma_start(x4[:, j], c2a3)
        else:
            xc3 = pers.tile([NB, S, DL], BF16)
            nc.sync.dma_start(xc3, c2a_out.rearrange("(i b) d -> b i d", b=NB))
            with tc.tile_pool(name="psumx", bufs=4, space="PSUM") as psumx:
                for i in range(S):
                    for h in range(2):
                        xr = psumx.tile([N, 512], F32, tag="xr")
                        nc.tensor.matmul(
                            xr, lhsT=rep_sb,
                            rhs=xc3[:, i, h * 512:(h + 1) * 512],
                            start=True, stop=True,
                        )
                        lo = i * DL + h * 512
                        if i % 2 == 0:
                            nc.vector.tensor_copy(x_all[:, lo:lo + 512], xr)
                        else:
                            nc.scalar.activation(x_all[:, lo:lo + 512], xr, AF.Copy)

        # ---- mining on this core's 12 rows ----
        pmax = pers.tile([NB, 8], F32)
        pidx = pers.tile([NB, 8], U32)
        nmax = pers.tile([NB, 8], F32)
        nidx = pers.tile([NB, 8], U32)
        s12 = pers.tile([NB, M], F32)
        nneg = pers.tile([NB, M], F32)
        if F_RS:
            nc.sync.dma_start(s12, rs_out)
            sub_eng.tensor_scalar_mul(nneg, s12, -1.0)
            nc.vector.max(pmax, s12)
            nc.vector.max_index(pidx, pmax, s12)
            nc.vector.max(nmax, nneg)
            nc.vector.max_index(nidx, nmax, nneg)
        else:
            s8 = pers.tile([N, M], F32, name="s8")
            nc.sync.dma_start(s8, rs_out)
            with tc.tile_pool(name="psumm", bufs=1, space="PSUM") as psumm:
                pm = psumm.tile([NB, M], F32)
                for b in range(3):
                    nc.tensor.matmul(
                        pm[:, b * 512:(b + 1) * 512], lhsT=fsel_sb,
                        rhs=s8[:, b * 512:(b + 1) * 512], start=True, stop=True,
                    )
                nc.vector.tensor_copy(s12, pm)
            sub_eng.tensor_scalar_mul(nneg, s12, -1.0)
            nc.vector.max(pmax, s12)
            nc.vector.max_index(pidx, pmax, s12)
            nc.vector.max(nmax, nneg)
            nc.vector.max_index(nidx, nmax, nneg)

        # ---- phase 2: winner-row gathers + local distances ----
        y96p = pers.tile([N, DL], BF16)
        y96n = pers.tile([N, DL], BF16)
        if F_BIGGATHER == 1:
            nc.gpsimd.indirect_dma_start(
                out=y96p.rearrange("(b j) d -> b j d", j=S), out_offset=None,
                in_=lf4,
                in_offset=bass.IndirectOffsetOnAxis(ap=pidx[:, 0:1], axis=0),
            )
            nc.gpsimd.indirect_dma_start(
                out=y96n.rearrange("(b j) d -> b j d", j=S), out_offset=None,
                in_=lf4,
                in_offset=bass.IndirectOffsetOnAxis(ap=nidx[:, 0:1], axis=0),
            )
        elif F_BIGGATHER == 2:
            # replicate each winner index to its 8 stripe partitions via a
            # tiny f32 matmul (entries 8.0 -> 8*idx, exact for idx < 2^19),
            # add the per-partition stripe offset, one 96-row gather per set
            rep8_sb = ins["rep8_sb"]
            joff_sb = ins["joff_sb"]
            with tc.tile_pool(name="psumg", bufs=1, space="PSUM") as psumg:
                for iset, (idx, y96) in enumerate(((pidx, y96p), (nidx, y96n))):
                    idf = pers.tile([NB, 1], F32, tag="idf", bufs=2)
                    nc.vector.tensor_copy(idf, idx[:, 0:1])
                    rp = psumg.tile([N, 1], F32, tag="rp", bufs=2)
                    nc.tensor.matmul(rp, lhsT=rep8_sb, rhs=idf,
                                     start=True, stop=True)
                    pj96 = pers.tile([N, 1], U32, tag="pj96", bufs=2)
                    nc.vector.tensor_tensor(
                        out=pj96, in0=rp, in1=joff_sb, op=ALU.add
                    )
                    nc.gpsimd.indirect_dma_start(
                        out=y96, out_offset=None, in_=lf_v,
                        in_offset=bass.IndirectOffsetOnAxis(ap=pj96, axis=0),
                    )
        else:
            y3p = y96p.rearrange("(b j) d -> b j d", j=S)
            y3n = y96n.rearrange("(b j) d -> b j d", j=S)
            for iset, (idx, y3) in enumerate(((pidx, y3p), (nidx, y3n))):
                for j in range(S):
                    pj = pers.tile([NB, 1], U32, tag="pj", bufs=4)
                    nc.vector.tensor_scalar(
                        out=pj, in0=idx[:, :1], scalar1=8, scalar2=j,
                        op0=ALU.mult, op1=ALU.add,
                    )
                    nc.gpsimd.indirect_dma_start(
                        out=y3[:, j, :], out_offset=None, in_=lf_v,
                        in_offset=bass.IndirectOffsetOnAxis(ap=pj, axis=0),
                    )

        d2t = pers.tile([N, 2 * S], F32)   # cols 0-7 pos, 8-15 neg
        mprod = None if F_TTR else pers.tile([N, S, DL], BF16)
        with tc.tile_pool(name="workp2", bufs=2) as work2:
            for i in range(S):
                xsl = x_all[:, i * DL:(i + 1) * DL]
                dtp = work2.tile([N, DL], BF16, tag="dtp")
                sub_eng.tensor_tensor(out=dtp, in0=y96p, in1=xsl, op=ALU.subtract)
                sqp = work2.tile([N, DL], BF16, tag="sqp")
                nc.scalar.activation(
                    sqp, dtp, AF.Square, accum_out=d2t[:, i:i + 1]
                )
                dtn = work2.tile([N, DL], BF16, tag="dtn")
                sub_eng.tensor_tensor(out=dtn, in0=y96n, in1=xsl, op=ALU.subtract)
                if F_TTR:
                    sqn = work2.tile([N, DL], BF16, tag="sqn")
                    nc.vector.tensor_tensor(
                        out=sqn, in0=dtn, in1=dtn, op=ALU.mult
                    )
                    nc.vector.tensor_reduce(
                        d2t[:, S + i:S + i + 1], sqn,
                        axis=mybir.AxisListType.X, op=ALU.add,
                    )
                else:
                    nc.vector.tensor_tensor(
                        out=mprod[:, i, :], in0=dtn, in1=dtn, op=ALU.mult
                    )
            if not F_TTR:
                nc.vector.tensor_reduce(
                    d2t[:, S:2 * S], mprod, axis=mybir.AxisListType.X, op=ALU.add
                )

            dcl = work2.tile([N, 2 * S], F32, tag="dcl", bufs=1)
            nc.vector.tensor_scalar_max(dcl, d2t, 1e-12)
            dsqr = work2.tile([N, 2 * S], F32, tag="dsqr", bufs=1)
            nc.scalar.activation(dsqr, dcl, AF.Sqrt)
            dsg = work2.tile([N, 2 * S], BF16, tag="dsg", bufs=1)
            nc.scalar.activation(dsg, dsqr, AF.Tanh, scale=0.5)
            # fold [96 (b,j), 16 (iset,i)] -> DP grid layout
            if F_SCAN:
                with tc.tile_pool(name="psf", bufs=1, space="PSUM") as psf_p:
                    psf = psf_p.tile([NB, 8 * 16], F32)
                    for j in range(S):
                        nc.tensor.matmul(
                            psf[:, j * 16:(j + 1) * 16],
                            lhsT=esel_sb[:, j * NB:(j + 1) * NB],
                            rhs=dsg, start=True, stop=True,
                        )
                    psf_v = psf.rearrange("p (j c) -> p c j", c=16)  # [12,16,8]
                    dg_v = dgrid.rearrange("p (i j) -> p i j", j=S)
                    nc.vector.tensor_copy(dg_v[0:NB], psf_v[:, 0:S, :])
                    nc.vector.tensor_copy(dg_v[32:32 + NB], psf_v[:, S:16, :])
            else:
                for iset in range(2):
                    with tc.tile_pool(name=f"psf{iset}", bufs=1,
                                      space="PSUM") as psf_p:
                        psf = psf_p.tile([NB, 64], F32)
                        for j in range(S):
                            nc.tensor.matmul(
                                psf[:, j * S:(j + 1) * S],
                                lhsT=esel_sb[:, j * NB:(j + 1) * NB],
                                rhs=dsg[:, iset * S:(iset + 1) * S],
                                start=True, stop=True,
                            )
                        dst = dD[iset * 32:iset * 32 + NB, 0:72].rearrange(
                            "p (i j) -> p j i", j=9
                        )[:, 1:9, :]
                        nc.vector.tensor_copy(
                            dst, psf.rearrange("p (j i) -> p j i", i=S)
                        )

        # ---- shortest-path DP ----
        if F_SCAN:
            for i in range(S):
                prev = bigrow if i == 0 else dp_sb[:, (i - 1) * S:i * S]
                nc.vector.tensor_tensor_scan(
                    out=dp_sb[:, i * S:(i + 1) * S],
                    data0=prev,
                    data1=dgrid[:, i * S:(i + 1) * S],
                    initial=0.0 if i == 0 else BIGDP,
                    op0=ALU.min, op1=ALU.add,
                )
            dp_last = dp_sb[:, 63:64]
        else:
            for kdiag in range(2, 17):
                lo = max(1, kdiag - 8)
                hi = min(8, kdiag - 1)
                cnt = hi - lo + 1
                f0 = 9 * lo + (kdiag - lo)
                t = pers.tile([64, 8], F32, tag="dptmp", bufs=2)
                nc.vector.tensor_tensor(
                    out=t[:, :cnt],
                    in0=dp[:, f0 - 9:f0 - 9 + 8 * (cnt - 1) + 1:8],
                    in1=dp[:, f0 - 1:f0 - 1 + 8 * (cnt - 1) + 1:8],
                    op=ALU.min,
                )
                nc.vector.tensor_tensor(
                    out=dp[:, f0:f0 + 8 * (cnt - 1) + 1:8],
                    in0=t[:, :cnt],
                    in1=dD[:, f0 - 9:f0 - 9 + 8 * (cnt - 1) + 1:8],
                    op=ALU.add,
                )
            dp_last = dp[:, 80:81]

        # ---- loss: relu(ap - an + margin), partial sum over 12 centers ----
        with tc.tile_pool(name="psum3", bufs=1, space="PSUM") as psum3:
            dps = psum3.tile([NB, 1], F32)
            nc.tensor.matmul(dps, lhsT=wsel_sb, rhs=dp_last, start=True, stop=True)
            r12 = pers.tile([NB, 1], F32)
            nc.vector.tensor_scalar(
                out=r12, in0=dps, scalar1=marg, scalar2=0.0,
                op0=ALU.add, op1=ALU.max,
            )
            lsum = psum3.tile([1, 1], F32)
            nc.tensor.matmul(lsum, lhsT=r12, rhs=ones_c, start=True, stop=True)
            out_sb = pers.tile([1, 1], F32)
            nc.vector.tensor_copy(out_sb, lsum)
        nc.sync.dma_start(outs["out"], out_sb)

        if DEBUG:
            nc.sync.dma_start(outs["dbg_pidx"], pidx)
            nc.sync.dma_start(outs["dbg_nidx"], nidx)
            nc.sync.dma_start(outs["dbg_d2t"], d2t)
            if F_SCAN:
                nc.sync.dma_start(outs["dbg_dgrid"], dgrid)
                nc.sync.dma_start(outs["dbg_dp"], dp_sb)
            nc.sync.dma_start(outs["dbg_s12"], s12)


def build_program():
    nc = bacc.Bacc(
        "TRN2", target_bir_lowering=False, debug=False,
        enable_asserts=False, num_devices=NCORES,
    )
    ins = {
        "fnat": nc.dram_tensor("fnat", [M, DPC], BF16, kind="ExternalInput").ap(),
        "ftT": nc.dram_tensor("ftT", [DPC, M], BF16, kind="ExternalInput").ap(),
        "lf": nc.dram_tensor("lf", [M, D], BF16, kind="ExternalInput").ap(),
        "um": nc.dram_tensor("um", [33, N], BF16, kind="ExternalInput").ap(),
        "vm": nc.dram_tensor("vm", [33, M], BF16, kind="ExternalInput").ap(),
        "wsel": nc.dram_tensor("wsel", [64, NB], F32, kind="ExternalInput").ap(),
        "ident": nc.dram_tensor("ident", [128, 128], F32, kind="ExternalInput").ap(),
        "esel": nc.dram_tensor("esel", [N, N], BF16, kind="ExternalInput").ap(),
        "aavg": nc.dram_tensor("aavg", [128, NT * N], BF16, kind="ExternalInput").ap(),
        "fsel": nc.dram_tensor("fsel", [N, NB], F32, kind="ExternalInput").ap(),
        "repsel": nc.dram_tensor("repsel", [NB, N], BF16, kind="ExternalInput").ap(),
        "rep8": nc.dram_tensor("rep8", [NB, N], F32, kind="ExternalInput").ap(),
        "joff": nc.dram_tensor("joff", [N, 1], F32, kind="ExternalInput").ap(),
    }
    outs = {"out": nc.dram_tensor("out", [1, 1], F32, kind="ExternalOutput").ap()}
    if DEBUG:
        outs["dbg_pidx"] = nc.dram_tensor(
            "dbg_pidx", [NB, 8], U32, kind="ExternalOutput").ap()
        outs["dbg_nidx"] = nc.dram_tensor(
            "dbg_nidx", [NB, 8], U32, kind="ExternalOutput").ap()
        outs["dbg_d2t"] = nc.dram_tensor(
            "dbg_d2t", [N, 2 * S], F32, kind="ExternalOutput").ap()
        if F_SCAN:
            outs["dbg_dgrid"] = nc.dram_tensor(
                "dbg_dgrid", [64, 64], F32, kind="ExternalOutput").ap()
            outs["dbg_dp"] = nc.dram_tensor(
                "dbg_dp", [64, 64], F32, kind="ExternalOutput").ap()
        outs["dbg_s12"] = nc.dram_tensor(
            "dbg_s12", [NB, M], F32, kind="ExternalOutput").ap()
    with tile.TileContext(nc) as tc:
        build_body(tc, outs, ins)
    nc.compile()
    return nc


def make_in_maps(feats, labels, local_features):
    bf16 = mybir.dt.np(BF16)
    feats = np.asarray(feats, dtype=np.float32).astype(bf16)
    labels = np.asarray(labels).astype(np.int64)
    lf_flat = np.ascontiguousarray(
        np.asarray(local_features, dtype=np.float32)
        .transpose(0, 2, 1).reshape(M, D)
    ).astype(bf16)
    anchors = labels[::K]  # [96]
    # mask factors: sum_r um[r,n]*vm[r,m] = -BIGM * (anchors[n] != labels[m])
    um = np.zeros((33, N), dtype=np.float32)
    vm = np.zeros((33, M), dtype=np.float32)
    um[0, :] = -BIGM
    vm[0, :] = 1.0
    for ell in range(L):
        um[1 + ell, :] = BIGM * (anchors == ell)
        vm[1 + ell, :] = (labels == ell)
    um = um.astype(bf16)
    vm = vm.astype(bf16)
    # fold selector: exchange-out row (NB*i + b) -> out row b (fallback)
    fsel = np.zeros((N, NB), dtype=np.float32)
    for i in range(NCORES):
        for b in range(NB):
            fsel[NB * i + b, b] = 1.0
    # replicate selector: center b -> partitions b*8..b*8+8 (fallback)
    repsel = np.zeros((NB, N), dtype=np.float32)
    for b in range(NB):
        repsel[b, b * S:(b + 1) * S] = 1.0
    repsel = repsel.astype(bf16)
    wsel = np.zeros((64, NB), dtype=np.float32)
    wsel[0:NB, :] = np.eye(NB)
    wsel[32:32 + NB, :] = -np.eye(NB)
    ident = np.eye(128, dtype=np.float32)
    esel = np.zeros((N, N), dtype=np.float32)
    for b in range(NB):
        for j in range(S):
            esel[b * S + j, j * NB + b] = 1.0
    esel = esel.astype(bf16)
    # packed chunk-averaging matrix: [128, t*96+n]
    aavg = np.zeros((M, N), dtype=np.float32)
    aavg[np.arange(M), np.arange(M) // K] = 1.0 / K
    aavg_p = np.ascontiguousarray(
        aavg.reshape(NT, 128, N).transpose(1, 0, 2).reshape(128, NT * N)
    ).astype(bf16)
    in_maps = []
    for c in range(NCORES):
        sl = feats[:, c * DPC:(c + 1) * DPC]
        in_maps.append({
            "fnat": np.ascontiguousarray(sl),
            "ftT": np.ascontiguousarray(sl.T),
            "lf": lf_flat,
            "um": um,
            "vm": vm,
            "wsel": wsel,
            "ident": ident,
            "esel": esel,
            "aavg": aavg_p,
            "fsel": fsel,
            "repsel": repsel,
            "rep8": 8.0 * repsel.astype(np.float32),
            "joff": np.tile(np.arange(S, dtype=np.float32), NB)[:, None],
        })
    return in_maps


_NC_CACHE = None


def _get_nc():
    global _NC_CACHE
    if _NC_CACHE is None:
        _NC_CACHE = build_program()
    return _NC_CACHE


def run(feats, labels, local_features, trace=False, **kwargs):
    nc = _get_nc()
    in_maps = make_in_maps(feats, labels, local_features)
    res = bass_utils.run_bass_kernel_spmd(
        nc, in_maps, core_ids=list(range(NCORES)), trace=trace, **kwargs
    )
    partial = sum(float(r["out"][0, 0]) for r in res.results)
    return np.float32(partial / N), res


def kernel(feats, labels, local_features):
    loss, _ = run(feats, labels, local_features)
    return loss


# revision 27
# speedup vs baseline: 1.5212x; 1.2122x over previous
"""Bass/Tile TRN2 kernel for nn_CenterAlignedTripletLoss (8-core SPMD).

Sharding (feature-parallel):
  Phase 1: feats sharded along the feature axis (8192 -> 1024 per core).
    Each core computes partial mining scores for all 96 centers over its
    slice:  s[n, m] = ||f_m||^2 - 2 c_n . f_m - BIGM*(1-is_pos[n,m])
    accumulated in PSUM (the label mask enters as a rank-33 matmul
    k-tile; partial norms as a rank-1 f32 k-tile; the row-constant
    ||c_n||^2 is omitted).  Norm tiles split ACT/DVE (tensor_tensor_
    reduce).  A tiny warmup AllToAll issued first absorbs the initial
    cross-core barrier + CC setup into the load phase.
  Exchange: Mesh AllToAll of bf16 center slices (each core gets
    full-feature centers for its 12-center block), then ReduceScatter
    (add) of the f32 partial scores so each core receives its 12 rows
    already summed -- no fold matmul.
  Mining: hardest positive = row argmax, hardest negative = row argmax
    of the negated matrix.
  Phase 2: one indirect-DMA gather per set (12 descriptors x 16KB: the
    winner's full 8-stripe local row, landing as (center,stripe)
    partition layout), stripe distances via GpSimd subtract + ACT
    square-accumulate (positive) / DVE tensor_tensor_reduce (negative),
    sqrt -> tanh(d/2) (sqrt table preloaded off-path), PE fold to the
    DP grid, then the AlignedReID shortest path as 8 row-wise
    tensor_tensor_scan(min, add) ops.  Partial loss summed on host.
"""

import os
import numpy as np
from contextlib import ExitStack

import concourse.bass as bass
import concourse.bacc as bacc
import concourse.tile as tile
from concourse import mybir
from concourse import bass_utils

F32 = mybir.dt.float32
BF16 = mybir.dt.bfloat16
U32 = mybir.dt.uint32
AF = mybir.ActivationFunctionType
ALU = mybir.AluOpType

NCORES = 8
M = 1536          # samples
D = 8192          # feature dim
DPC = D // NCORES # 1024 features per core
N = 96            # centers
NB = N // NCORES  # 12 centers per core
K = 16            # samples per chunk
S = 8             # stripes
DL = 1024         # local feature dim per stripe
L = 32            # label count
MARGIN = 0.3
BIGM = 16384.0    # per-core mask magnitude (sums to 2^17 after RS)
BIGDP = 1.0e6     # DP "no predecessor" value
RG = [list(range(NCORES))]
NT = M // 128     # 12 sample tiles

DEBUG = False     # add intermediate ExternalOutputs (test harness only)


def _flag(name, default="1"):
    return os.environ.get(name, default) == "1"


F_WARMUP = _flag("K_WARMUP", "0")  # tiny warmup collective first (net-zero)
F_RS = _flag("K_RS", "0")          # ReduceScatter scores (slower than A2A+fold)
# gather mode: 0 = 16 per-stripe gathers, 1 = 16KB-elem (2 gathers),
# 2 = 96-row batched (2 gathers + index replicate matmul)
F_BIGGATHER = int(os.environ.get("K_BIGGATHER", "2"))
F_SCAN = _flag("K_SCAN")           # tensor_tensor_scan DP (else antidiagonal)
F_XDMA = _flag("K_XDMA")           # x_all replicate via DMA (else PE matmuls)
F_TTR = _flag("K_TTR")             # per-stripe neg reduce (else monolithic)
F_GPS = _flag("K_GPS", "0")        # gpsimd for subtracts (3x slower than DVE)
F_SFIRST = _flag("K_SFIRST")       # scores A2A issued before centers A2A
# how many neg stripes use DVE mult+reduce instead of ACT square (balance)
N_DVE_NEG = int(os.environ.get("K_DVE_NEG", "4"))
# how many subtracts (of 16) go to GpSimd (it is ~3x slower but otherwise idle)
N_GPS_SUB = int(os.environ.get("K_GPS_SUB", "3"))


def build_body(tc, outs, ins):
    nc = tc.nc
    fnat = ins["fnat"]          # [M, DPC] natural feature slice (bf16)
    ftT = ins["ftT"]            # [DPC, M] pre-transposed slice (bf16)
    lf = ins["lf"]              # [M, D] local_features (stripe, d) bf16
    um = ins["um"]              # [33, 96] mask lhsT (bf16)
    vm = ins["vm"]              # [33, M] mask rhs (bf16)
    wsel = ins["wsel"]          # [64, NB] +I / -I rows (f32)
    ident = ins["ident"]        # [128, 128] identity (f32)
    esel = ins["esel"]          # [96, 96] grid-fold selection (bf16)
    aavg = ins["aavg"]          # [128, NT*N] packed chunk-avg matrix bf16
    fsel = ins["fsel"]          # [96, NB] fold selector (f32, fallback)
    repsel = ins["repsel"]      # [NB, 96] replicate selector (bf16, fallback)

    lf4 = lf.rearrange("m (j d) -> m j d", d=DL)   # [1536, 8, 1024]
    lf_v = lf.rearrange("m (j d) -> (m j) d", d=DL)

    sub_eng = nc.gpsimd if F_GPS else nc.vector

    with ExitStack() as ctx:
        const = ctx.enter_context(tc.tile_pool(name="const", bufs=1))
        pers = ctx.enter_context(tc.tile_pool(name="pers", bufs=1))
        dram = ctx.enter_context(tc.tile_pool(name="dram", bufs=1, space="DRAM"))

        # DRAM bounce buffers for collectives
        warm_in = dram.tile([NCORES, 1], F32)
        warm_out = dram.tile([NCORES, 1], F32)
        c2a_in = dram.tile([N, DPC], BF16)
        c2a_out = dram.tile([N, DPC], BF16)
        rs_in = dram.tile([N, M], F32)
        rs_out = dram.tile([NB if F_RS else N, M], F32)
        nrow_d = dram.tile([NT, 128], F32)

        # ---- warmup collective first: absorbs the initial cross-core
        # barrier + CC ring setup while the loads/compute run ----
        if F_WARMUP:
            nc.gpsimd.collective_compute(
                "AllToAll", ALU.bypass, replica_groups=RG,
                ins=[warm_in.opt()], outs=[warm_out.opt()],
            )

        # ---- big loads first so queues fill immediately ----
        nt_sb = [const.tile([128, DPC], BF16, name=f"nt{t}") for t in range(NT)]
        for t in range(NT):
            nc.sync.dma_start(nt_sb[t], fnat[t * 128:(t + 1) * 128, :])
        ftT_sb = [const.tile([128, M], BF16, name=f"ftT{kd}") for kd in range(8)]
        for kd in range(8):
            nc.sync.dma_start(ftT_sb[kd], ftT[kd * 128:(kd + 1) * 128, :])
        ident_sb = const.tile([128, 128], F32)
        nc.sync.dma_start(ident_sb, ident)
        wsel_sb = const.tile([64, NB], F32)
        nc.sync.dma_start(wsel_sb, wsel)
        esel_sb = const.tile([N, N], BF16)
        nc.sync.dma_start(esel_sb, esel)
        um_sb = const.tile([33, N], BF16)
        nc.sync.dma_start(um_sb, um)
        vm_sb = const.tile([33, M], BF16)
        nc.sync.dma_start(vm_sb, vm)
        if not F_RS:
            fsel_sb = const.tile([N, NB], F32)
            nc.sync.dma_start(fsel_sb, fsel)
            nfsel_sb = const.tile([N, NB], F32)
            nc.sync.dma_start(nfsel_sb, ins["nfsel"])
        if not F_XDMA:
            rep_sb = const.tile([NB, N], BF16)
            nc.sync.dma_start(rep_sb, repsel)
        if F_BIGGATHER == 2:
            rep8_sb = const.tile([NB, N], F32)
            nc.sync.dma_start(rep8_sb, ins["rep8"])
            joff_sb = const.tile([N, 1], F32)
            nc.sync.dma_start(joff_sb, ins["joff"])
            ins["rep8_sb"] = rep8_sb
            ins["joff_sb"] = joff_sb
        ones_c = const.tile([NB, 1], F32)
        nc.vector.memset(ones_c, 1.0)
        marg = const.tile([NB, 1], F32)
        nc.vector.memset(marg, MARGIN)
        idx0 = const.tile([2, 1], U32)
        nc.vector.memset(idx0, 0)
        a_sb = const.tile([128, NT * N], BF16)
        nc.sync.dma_start(a_sb, aavg)

        ctr_cm = pers.tile([N, DPC], F32)   # center-major centers slice (f32)
        ctr_bf = pers.tile([N, DPC], BF16)  # bf16 copy for the exchange
        x_all = pers.tile([N, D], BF16)     # centers replicated over stripes
        nrow = pers.tile([1, M], F32)       # partial sample norms row
        normcol = pers.tile([128, NT], F32)
        normT_sb = pers.tile([NT, 128], F32)

        # phase-2 grids
        if F_SCAN:
            dgrid = pers.tile([64, 64], F32)    # [b(+32 for neg), 8i x 8j]
            nc.vector.memset(dgrid, 0.0)
            bigrow = pers.tile([64, S], F32)
            nc.vector.memset(bigrow, BIGDP)
            dp_sb = pers.tile([64, 64], F32)    # DP rows
        else:
            dD = pers.tile([64, 81], F32)
            nc.vector.memset(dD, 0.0)
            dp = pers.tile([64, 81], F32)
            nc.vector.memset(dp, BIGDP)
            nc.vector.memset(dp[:, 1:2], 0.0)

        # ---- phase 1 ----
        with tc.tile_pool(name="psumc", bufs=1, space="PSUM") as psumc, \
             tc.tile_pool(name="psums", bufs=1, space="PSUM") as psums, \
             tc.tile_pool(name="psumt", bufs=2, space="PSUM") as psumt, \
             tc.tile_pool(name="workp1", bufs=2) as work:
            ps_c = [psumc.tile([N, 512], F32, name=f"ps_c{h}") for h in range(2)]
            s_ps = [psums.tile([N, 512], F32, name=f"s_ps{b}") for b in range(3)]
            for t in range(NT):
                for h in range(2):
                    nc.tensor.matmul(
                        ps_c[h], lhsT=a_sb[:, t * N:(t + 1) * N],
                        rhs=nt_sb[t][:, h * 512:(h + 1) * 512],
                        start=(t == 0), stop=(t == NT - 1),
                    )
                # partial sample norms: split ACT / DVE to halve the chain
                if F_TTR and t % 2 == 1:
                    sqv = work.tile([128, DPC], BF16, tag="sqv")
                    nc.vector.tensor_tensor(
                        out=sqv, in0=nt_sb[t], in1=nt_sb[t], op=ALU.mult
                    )
                    nc.vector.tensor_reduce(
                        normcol[:, t:t + 1], sqv,
                        axis=mybir.AxisListType.X, op=ALU.add,
                    )
                else:
                    sqd = work.tile([128, DPC], F32, tag="sqd")
                    nc.scalar.activation(
                        sqd, nt_sb[t], AF.Square, accum_out=normcol[:, t:t + 1]
                    )
            # centers psum -> f32 (DVE, for transposes) + bf16 (ACT, exchange)
            for h in range(2):
                nc.vector.tensor_copy(ctr_cm[:, h * 512:(h + 1) * 512], ps_c[h])
                nc.scalar.activation(
                    ctr_bf[:, h * 512:(h + 1) * 512], ps_c[h], AF.Copy
                )
            nc.sync.dma_start(c2a_in, ctr_bf)
            if not F_SFIRST:
                nc.gpsimd.collective_compute(
                    "AllToAll", ALU.bypass, replica_groups=RG,
                    ins=[c2a_in.opt()], outs=[c2a_out.opt()],
                )

            # mask k-tile first (only needs consts)
            for b in range(3):
                nc.tensor.matmul(
                    s_ps[b], lhsT=um_sb, rhs=vm_sb[:, b * 512:(b + 1) * 512],
                    start=True, stop=False,
                )
            # ctrT blocks scaled by -2 for the dot matmuls
            ctrT2 = []
            for kd in range(8):
                tpc = psumt.tile([128, N], F32, tag="tp")
                nc.tensor.transpose(
                    tpc, ctr_cm[:, kd * 128:(kd + 1) * 128], ident_sb[:N, :N]
                )
                c2 = work.tile([128, N], BF16, tag=f"ctrT{kd}", bufs=1)
                nc.vector.tensor_scalar_mul(c2, tpc, -2.0)
                ctrT2.append(c2)
            # norm row [1, 1536]: one transpose + DRAM-bounced rearrange
            ones_n = const.tile([1, N], F32)
            nc.vector.memset(ones_n, 1.0)
            tpn = psumt.tile([NT, 128], F32, tag="tpn", bufs=1)
            nc.tensor.transpose(tpn, normcol, ident_sb)
            nc.vector.tensor_copy(normT_sb, tpn)
            nc.sync.dma_start(nrow_d, normT_sb)
            nc.sync.dma_start(nrow, nrow_d.flatten().unsqueeze(0))
            # scores: accumulate -2 c.f over 8 k-tiles, then the norm row
            for kd in range(8):
                for b in range(3):
                    nc.tensor.matmul(
                        s_ps[b], lhsT=ctrT2[kd],
                        rhs=ftT_sb[kd][:, b * 512:(b + 1) * 512],
                        start=False, stop=False,
                    )
            for b in range(3):
                nc.tensor.matmul(
                    s_ps[b], lhsT=ones_n, rhs=nrow[:, b * 512:(b + 1) * 512],
                    start=False, stop=True,
                )
            s_sb = pers.tile([N, M], F32)
            for b in range(3):
                nc.vector.tensor_copy(s_sb[:, b * 512:(b + 1) * 512], s_ps[b])
            nc.sync.dma_start(rs_in, s_sb)
            # dummy indirect gather: absorbs the SWDGE drain before the
            # real gathers land on the critical path
            dscr = pers.tile([2, 64], BF16)
            nc.gpsimd.indirect_dma_start(
                out=dscr, out_offset=None, in_=lf_v[:, :64],
                in_offset=bass.IndirectOffsetOnAxis(ap=idx0, axis=0),
            )
            if F_RS:
                nc.gpsimd.collective_compute(
                    "ReduceScatter", ALU.add, replica_groups=RG,
                    ins=[rs_in.opt()], outs=[rs_out.opt()],
                )
            else:
                nc.gpsimd.collective_compute(
                    "AllToAll", ALU.bypass, replica_groups=RG,
                    ins=[rs_in.opt()], outs=[rs_out.opt()],
                )
            if F_SFIRST:
                nc.gpsimd.collective_compute(
                    "AllToAll", ALU.bypass, replica_groups=RG,
                    ins=[c2a_in.opt()], outs=[c2a_out.opt()],
                )

        # preload the Sqrt activation table (slot beside Square) while the
        # collectives are on the wire, so phase 2 pays no sqrt table load
        sq_warm = pers.tile([1, S], F32)
        nc.vector.memset(sq_warm, 1.0)
        sq_warm2 = pers.tile([1, S], F32)
        nc.scalar.activation(sq_warm2, sq_warm, AF.Sqrt)

        # ---- x_all: replicate centers over stripe partitions ----
        # (runs while the scores exchange is on the wire)
        if F_XDMA:
            c2a3 = c2a_out.rearrange("(i b) d -> b i d", b=NB)   # [12, 8, 1024]
            x4 = x_all.rearrange("(b j) (i d) -> b j i d", j=S, d=DL)
            for j in range(S):
                nc.sync.dma_start(x4[:, j], c2a3)
        else:
            xc3 = pers.tile([NB, S, DL], BF16)
            nc.sync.dma_start(xc3, c2a_out.rearrange("(i b) d -> b i d", b=NB))
            with tc.tile_pool(name="psumx", bufs=4, space="PSUM") as psumx:
                for i in range(S):
                    for h in range(2):
                        xr = psumx.tile([N, 512], F32, tag="xr")
                        nc.tensor.matmul(
                            xr, lhsT=rep_sb,
                            rhs=xc3[:, i, h * 512:(h + 1) * 512],
                            start=True, stop=True,
                        )
                        lo = i * DL + h * 512
                        if i % 2 == 0:
                            nc.vector.tensor_copy(x_all[:, lo:lo + 512], xr)
                        else:
                            nc.scalar.activation(x_all[:, lo:lo + 512], xr, AF.Copy)

        # ---- mining on this core's 12 rows ----
        pmax = pers.tile([NB, 8], F32)
        pidx = pers.tile([NB, 8], U32)
        nmax = pers.tile([NB, 8], F32)
        nidx = pers.tile([NB, 8], U32)
        s12 = pers.tile([NB, M], F32, name="s12") if (F_RS or DEBUG) else None
        mine_ctx = ExitStack()
        if F_RS:
            nc.sync.dma_start(s12, rs_out)
            nneg = pers.tile([NB, M], F32)
            nc.vector.tensor_scalar_mul(nneg, s12, -1.0)
            nc.vector.max(pmax, s12)
            nc.vector.max_index(pidx, pmax, s12)
            nc.vector.max(nmax, nneg)
            nc.vector.max_index(nidx, nmax, nneg)
        else:
            # dual fold: +fsel and -fsel matmuls give the positive-mining
            # matrix and its negation straight in PSUM (no DVE negate)
            s8 = pers.tile([N, M], F32, name="s8")
            for b in range(3):
                nc.sync.dma_start(
                    s8[:, b * 512:(b + 1) * 512],
                    rs_out[:, b * 512:(b + 1) * 512],
                )
            psumm = mine_ctx.enter_context(
                tc.tile_pool(name="psumm", bufs=1, space="PSUM"))
            pm = psumm.tile([NB, M], F32)
            nm = psumm.tile([NB, M], F32)
            for b in range(3):
                nc.tensor.matmul(
                    pm[:, b * 512:(b + 1) * 512], lhsT=fsel_sb,
                    rhs=s8[:, b * 512:(b + 1) * 512], start=True, stop=True,
                )
                nc.tensor.matmul(
                    nm[:, b * 512:(b + 1) * 512], lhsT=nfsel_sb,
                    rhs=s8[:, b * 512:(b + 1) * 512], start=True, stop=True,
                )
            nc.vector.max(pmax, pm)
            nc.vector.max_index(pidx, pmax, pm)
            nc.vector.max(nmax, nm)
            nc.vector.max_index(nidx, nmax, nm)
            if DEBUG:
                nc.vector.tensor_copy(s12, pm)

        # ---- phase 2: winner-row gathers + local distances ----
        y96p = pers.tile([N, DL], BF16)
        y96n = pers.tile([N, DL], BF16)
        if F_BIGGATHER == 1:
            nc.gpsimd.indirect_dma_start(
                out=y96p.rearrange("(b j) d -> b j d", j=S), out_offset=None,
                in_=lf4,
                in_offset=bass.IndirectOffsetOnAxis(ap=pidx[:, 0:1], axis=0),
            )
            nc.gpsimd.indirect_dma_start(
                out=y96n.rearrange("(b j) d -> b j d", j=S), out_offset=None,
                in_=lf4,
                in_offset=bass.IndirectOffsetOnAxis(ap=nidx[:, 0:1], axis=0),
            )
        elif F_BIGGATHER == 2:
            # replicate each winner index to its 8 stripe partitions via a
            # tiny f32 matmul (entries 8.0 -> 8*idx, exact for idx < 2^19),
            # add the per-partition stripe offset, one 96-row gather per set
            rep8_sb = ins["rep8_sb"]
            joff_sb = ins["joff_sb"]
            with tc.tile_pool(name="psumg", bufs=1, space="PSUM") as psumg:
                for iset, (idx, y96) in enumerate(((pidx, y96p), (nidx, y96n))):
                    idf = pers.tile([NB, 1], F32, tag="idf", bufs=2)
                    nc.vector.tensor_copy(idf, idx[:, 0:1])
                    rp = psumg.tile([N, 1], F32, tag="rp", bufs=2)
                    nc.tensor.matmul(rp, lhsT=rep8_sb, rhs=idf,
                                     start=True, stop=True)
                    pj96 = pers.tile([N, 1], U32, tag="pj96", bufs=2)
                    nc.vector.tensor_tensor(
                        out=pj96, in0=rp, in1=joff_sb, op=ALU.add
                    )
                    nc.gpsimd.indirect_dma_start(
                        out=y96, out_offset=None, in_=lf_v,
                        in_offset=bass.IndirectOffsetOnAxis(ap=pj96, axis=0),
                    )
        else:
            y3p = y96p.rearrange("(b j) d -> b j d", j=S)
            y3n = y96n.rearrange("(b j) d -> b j d", j=S)
            for iset, (idx, y3) in enumerate(((pidx, y3p), (nidx, y3n))):
                for j in range(S):
                    pj = pers.tile([NB, 1], U32, tag="pj", bufs=4)
                    nc.vector.tensor_scalar(
                        out=pj, in0=idx[:, :1], scalar1=8, scalar2=j,
                        op0=ALU.mult, op1=ALU.add,
                    )
                    nc.gpsimd.indirect_dma_start(
                        out=y3[:, j, :], out_offset=None, in_=lf_v,
                        in_offset=bass.IndirectOffsetOnAxis(ap=pj, axis=0),
                    )
        mine_ctx.close()

        d2t = pers.tile([N, 2 * S], F32)   # cols 0-7 pos, 8-15 neg
        with tc.tile_pool(name="workp2", bufs=3) as work2:
            # positive diffs first so ACT can chew squares continuously
            dtp = []
            for i in range(S):
                xsl = x_all[:, i * DL:(i + 1) * DL]
                dt = work2.tile([N, DL], BF16, tag=f"dtp{i}", bufs=1)
                nc.vector.tensor_tensor(out=dt, in0=y96p, in1=xsl,
                                        op=ALU.subtract)
                dtp.append(dt)
            for i in range(S):
                sqp = work2.tile([N, DL], BF16, tag="sqp")
                nc.scalar.activation(
                    sqp, dtp[i], AF.Square, accum_out=d2t[:, i:i + 1]
                )
            # negatives: subtracts split DVE/GpSimd, square-sums split ACT/DVE
            for i in range(S):
                xsl = x_all[:, i * DL:(i + 1) * DL]
                dtn = work2.tile([N, DL], BF16, tag="dtn")
                eng = nc.gpsimd if i < N_GPS_SUB else nc.vector
                eng.tensor_tensor(out=dtn, in0=y96n, in1=xsl, op=ALU.subtract)
                if i < S - N_DVE_NEG:
                    sqn = work2.tile([N, DL], BF16, tag="sqn")
                    nc.scalar.activation(
                        sqn, dtn, AF.Square, accum_out=d2t[:, S + i:S + i + 1]
                    )
                else:
                    sqn = work2.tile([N, DL], BF16, tag="sqn")
                    nc.vector.tensor_tensor(
                        out=sqn, in0=dtn, in1=dtn, op=ALU.mult
                    )
                    nc.vector.tensor_reduce(
                        d2t[:, S + i:S + i + 1], sqn,
                        axis=mybir.AxisListType.X, op=ALU.add,
                    )

            dcl = work2.tile([N, 2 * S], F32, tag="dcl", bufs=1)
            nc.vector.tensor_scalar_max(dcl, d2t, 1e-12)
            dsqr = work2.tile([N, 2 * S], F32, tag="dsqr", bufs=1)
            nc.scalar.activation(dsqr, dcl, AF.Sqrt)
            dsg = work2.tile([N, 2 * S], BF16, tag="dsg", bufs=1)
            nc.scalar.activation(dsg, dsqr, AF.Tanh, scale=0.5)
            # fold [96 (b,j), 16 (iset,i)] -> DP grid layout
            if F_SCAN:
                with tc.tile_pool(name="psf", bufs=1, space="PSUM") as psf_p:
                    psf = psf_p.tile([NB, 8 * 16], F32)
                    for j in range(S):
                        nc.tensor.matmul(
                            psf[:, j * 16:(j + 1) * 16],
                            lhsT=esel_sb[:, j * NB:(j + 1) * NB],
                            rhs=dsg, start=True, stop=True,
                        )
                    psf_v = psf.rearrange("p (j c) -> p c j", c=16)  # [12,16,8]
                    dg_v = dgrid.rearrange("p (i j) -> p i j", j=S)
                    nc.vector.tensor_copy(dg_v[0:NB], psf_v[:, 0:S, :])
                    nc.vector.tensor_copy(dg_v[32:32 + NB], psf_v[:, S:16, :])
            else:
                for iset in range(2):
                    with tc.tile_pool(name=f"psf{iset}", bufs=1,
                                      space="PSUM") as psf_p:
                        psf = psf_p.tile([NB, 64], F32)
                        for j in range(S):
                            nc.tensor.matmul(
                                psf[:, j * S:(j + 1) * S],
                                lhsT=esel_sb[:, j * NB:(j + 1) * NB],
                                rhs=dsg[:, iset * S:(iset + 1) * S],
                                start=True, stop=True,
                            )
                        dst = dD[iset * 32:iset * 32 + NB, 0:72].rearrange(
                            "p (i j) -> p j i", j=9
                        )[:, 1:9, :]
                        nc.vector.tensor_copy(
                            dst, psf.rearrange("p (j i) -> p j i", i=S)
                        )

        # ---- shortest-path DP ----
        if F_SCAN:
            for i in range(S):
                prev = bigrow if i == 0 else dp_sb[:, (i - 1) * S:i * S]
                nc.vector.tensor_tensor_scan(
                    out=dp_sb[:, i * S:(i + 1) * S],
                    data0=prev,
                    data1=dgrid[:, i * S:(i + 1) * S],
                    initial=0.0 if i == 0 else BIGDP,
                    op0=ALU.min, op1=ALU.add,
                )
            dp_last = dp_sb[:, 63:64]
        else:
            for kdiag in range(2, 17):
                lo = max(1, kdiag - 8)
                hi = min(8, kdiag - 1)
                cnt = hi - lo + 1
                f0 = 9 * lo + (kdiag - lo)
                t = pers.tile([64, 8], F32, tag="dptmp", bufs=2)
                nc.vector.tensor_tensor(
                    out=t[:, :cnt],
                    in0=dp[:, f0 - 9:f0 - 9 + 8 * (cnt - 1) + 1:8],
                    in1=dp[:, f0 - 1:f0 - 1 + 8 * (cnt - 1) + 1:8],
                    op=ALU.min,
                )
                nc.vector.tensor_tensor(
                    out=dp[:, f0:f0 + 8 * (cnt - 1) + 1:8],
                    in0=t[:, :cnt],
                    in1=dD[:, f0 - 9:f0 - 9 + 8 * (cnt - 1) + 1:8],
                    op=ALU.add,
                )
            dp_last = dp[:, 80:81]

        # ---- loss: relu(ap - an + margin), partial sum over 12 centers ----
        with tc.tile_pool(name="psum3", bufs=1, space="PSUM") as psum3:
            dps = psum3.tile([NB, 1], F32)
            nc.tensor.matmul(dps, lhsT=wsel_sb, rhs=dp_last, start=True, stop=True)
            r12 = pers.tile([NB, 1], F32)
            nc.vector.tensor_scalar(
                out=r12, in0=dps, scalar1=marg, scalar2=0.0,
                op0=ALU.add, op1=ALU.max,
            )
            lsum = psum3.tile([1, 1], F32)
            nc.tensor.matmul(lsum, lhsT=r12, rhs=ones_c, start=True, stop=True)
            out_sb = pers.tile([1, 1], F32)
            nc.vector.tensor_copy(out_sb, lsum)
        nc.sync.dma_start(outs["out"], out_sb)

        if DEBUG:
            nc.sync.dma_start(outs["dbg_pidx"], pidx)
            nc.sync.dma_start(outs["dbg_nidx"], nidx)
            nc.sync.dma_start(outs["dbg_d2t"], d2t)
            if F_SCAN:
                nc.sync.dma_start(outs["dbg_dgrid"], dgrid)
                nc.sync.dma_start(outs["dbg_dp"], dp_sb)
            nc.sync.dma_start(outs["dbg_s12"], s12)


def build_program():
    nc = bacc.Bacc(
        "TRN2", target_bir_lowering=False, debug=False,
        enable_asserts=False, num_devices=NCORES,
    )
    ins = {
        "fnat": nc.dram_tensor("fnat", [M, DPC], BF16, kind="ExternalInput").ap(),
        "ftT": nc.dram_tensor("ftT", [DPC, M], BF16, kind="ExternalInput").ap(),
        "lf": nc.dram_tensor("lf", [M, D], BF16, kind="ExternalInput").ap(),
        "um": nc.dram_tensor("um", [33, N], BF16, kind="ExternalInput").ap(),
        "vm": nc.dram_tensor("vm", [33, M], BF16, kind="ExternalInput").ap(),
        "wsel": nc.dram_tensor("wsel", [64, NB], F32, kind="ExternalInput").ap(),
        "ident": nc.dram_tensor("ident", [128, 128], F32, kind="ExternalInput").ap(),
        "esel": nc.dram_tensor("esel", [N, N], BF16, kind="ExternalInput").ap(),
        "aavg": nc.dram_tensor("aavg", [128, NT * N], BF16, kind="ExternalInput").ap(),
        "fsel": nc.dram_tensor("fsel", [N, NB], F32, kind="ExternalInput").ap(),
        "repsel": nc.dram_tensor("repsel", [NB, N], BF16, kind="ExternalInput").ap(),
        "rep8": nc.dram_tensor("rep8", [NB, N], F32, kind="ExternalInput").ap(),
        "joff": nc.dram_tensor("joff", [N, 1], F32, kind="ExternalInput").ap(),
        "nfsel": nc.dram_tensor("nfsel", [N, NB], F32, kind="ExternalInput").ap(),
    }
    outs = {"out": nc.dram_tensor("out", [1, 1], F32, kind="ExternalOutput").ap()}
    if DEBUG:
        outs["dbg_pidx"] = nc.dram_tensor(
            "dbg_pidx", [NB, 8], U32, kind="ExternalOutput").ap()
        outs["dbg_nidx"] = nc.dram_tensor(
            "dbg_nidx", [NB, 8], U32, kind="ExternalOutput").ap()
        outs["dbg_d2t"] = nc.dram_tensor(
            "dbg_d2t", [N, 2 * S], F32, kind="ExternalOutput").ap()
        if F_SCAN:
            outs["dbg_dgrid"] = nc.dram_tensor(
                "dbg_dgrid", [64, 64], F32, kind="ExternalOutput").ap()
            outs["dbg_dp"] = nc.dram_tensor(
                "dbg_dp", [64, 64], F32, kind="ExternalOutput").ap()
        outs["dbg_s12"] = nc.dram_tensor(
            "dbg_s12", [NB, M], F32, kind="ExternalOutput").ap()
    with tile.TileContext(nc) as tc:
        build_body(tc, outs, ins)
    nc.compile()
    return nc


def make_in_maps(feats, labels, local_features):
    bf16 = mybir.dt.np(BF16)
    feats = np.asarray(feats, dtype=np.float32).astype(bf16)
    labels = np.asarray(labels).astype(np.int64)
    lf_flat = np.ascontiguousarray(
        np.asarray(local_features, dtype=np.float32)
        .transpose(0, 2, 1).reshape(M, D)
    ).astype(bf16)
    anchors = labels[::K]  # [96]
    # mask factors: sum_r um[r,n]*vm[r,m] = -BIGM * (anchors[n] != labels[m])
    um = np.zeros((33, N), dtype=np.float32)
    vm = np.zeros((33, M), dtype=np.float32)
    um[0, :] = -BIGM
    vm[0, :] = 1.0
    for ell in range(L):
        um[1 + ell, :] = BIGM * (anchors == ell)
        vm[1 + ell, :] = (labels == ell)
    um = um.astype(bf16)
    vm = vm.astype(bf16)
    # fold selector: exchange-out row (NB*i + b) -> out row b (fallback)
    fsel = np.zeros((N, NB), dtype=np.float32)
    for i in range(NCORES):
        for b in range(NB):
            fsel[NB * i + b, b] = 1.0
    # replicate selector: center b -> partitions b*8..b*8+8 (fallback)
    repsel = np.zeros((NB, N), dtype=np.float32)
    for b in range(NB):
        repsel[b, b * S:(b + 1) * S] = 1.0
    repsel = repsel.astype(bf16)
    wsel = np.zeros((64, NB), dtype=np.float32)
    wsel[0:NB, :] = np.eye(NB)
    wsel[32:32 + NB, :] = -np.eye(NB)
    ident = np.eye(128, dtype=np.float32)
    esel = np.zeros((N, N), dtype=np.float32)
    for b in range(NB):
        for j in range(S):
            esel[b * S + j, j * NB + b] = 1.0
    esel = esel.astype(bf16)
    # packed chunk-averaging matrix: [128, t*96+n]
    aavg = np.zeros((M, N), dtype=np.float32)
    aavg[np.arange(M), np.arange(M) // K] = 1.0 / K
    aavg_p = np.ascontiguousarray(
        aavg.reshape(NT, 128, N).transpose(1, 0, 2).reshape(128, NT * N)
    ).astype(bf16)
    in_maps = []
    for c in range(NCORES):
        sl = feats[:, c * DPC:(c + 1) * DPC]
        in_maps.append({
            "fnat": np.ascontiguousarray(sl),
            "ftT": np.ascontiguousarray(sl.T),
            "lf": lf_flat,
            "um": um,
            "vm": vm,
            "wsel": wsel,
            "ident": ident,
            "esel": esel,
            "aavg": aavg_p,
            "fsel": fsel,
            "repsel": repsel,
            "rep8": 8.0 * repsel.astype(np.float32),
            "joff": np.tile(np.arange(S, dtype=np.float32), NB)[:, None],
            "nfsel": -fsel,
        })
    return in_maps


_NC_CACHE = None


def _get_nc():
    global _NC_CACHE
    if _NC_CACHE is None:
        _NC_CACHE = build_program()
    return _NC_CACHE


def run(feats, labels, local_features, trace=False, **kwargs):
    nc = _get_nc()
    in_maps = make_in_maps(feats, labels, local_features)
    res = bass_utils.run_bass_kernel_spmd(
        nc, in_maps, core_ids=list(range(NCORES)), trace=trace, **kwargs
    )
    partial = sum(float(r["out"][0, 0]) for r in res.results)
    return np.float32(partial / N), res


def kernel(feats, labels, local_features):
    loss, _ = run(feats, labels, local_features)
    return loss


# revision 37
# speedup vs baseline: 1.6494x; 1.0843x over previous
"""Bass/Tile TRN2 kernel for nn_CenterAlignedTripletLoss (8-core SPMD).

Sharding (feature-parallel):
  Phase 1: feats sharded along the feature axis (8192 -> 1024 per core).
    Each core computes partial mining scores for all 96 centers over its
    slice:  s[n, m] = ||f_m||^2 - 2 c_n . f_m - BIGM*(1-is_pos[n,m])
    accumulated in PSUM (the label mask enters as a rank-33 matmul
    k-tile; partial norms as a rank-1 f32 k-tile; the row-constant
    ||c_n||^2 is omitted).  Norm tiles split ACT/DVE (tensor_tensor_
    reduce).  A tiny warmup AllToAll issued first absorbs the initial
    cross-core barrier + CC setup into the load phase.
  Exchange: Mesh AllToAll of bf16 center slices (each core gets
    full-feature centers for its 12-center block), then ReduceScatter
    (add) of the f32 partial scores so each core receives its 12 rows
    already summed -- no fold matmul.
  Mining: hardest positive = row argmax, hardest negative = row argmax
    of the negated matrix.
  Phase 2: one indirect-DMA gather per set (12 descriptors x 16KB: the
    winner's full 8-stripe local row, landing as (center,stripe)
    partition layout), stripe distances via GpSimd subtract + ACT
    square-accumulate (positive) / DVE tensor_tensor_reduce (negative),
    sqrt -> tanh(d/2) (sqrt table preloaded off-path), PE fold to the
    DP grid, then the AlignedReID shortest path as 8 row-wise
    tensor_tensor_scan(min, add) ops.  Partial loss summed on host.
"""

import os
import numpy as np
from contextlib import ExitStack

import concourse.bass as bass
import concourse.bacc as bacc
import concourse.tile as tile
from concourse import mybir
from concourse import bass_utils

F32 = mybir.dt.float32
BF16 = mybir.dt.bfloat16
U32 = mybir.dt.uint32
AF = mybir.ActivationFunctionType
ALU = mybir.AluOpType

NCORES = 8
M = 1536          # samples
D = 8192          # feature dim
DPC = D // NCORES # 1024 features per core
N = 96            # centers
NB = N // NCORES  # 12 centers per core
K = 16            # samples per chunk
S = 8             # stripes
DL = 1024         # local feature dim per stripe
L = 32            # label count
MARGIN = 0.3
BIGM = 512.0      # per-core mask magnitude (sums to 4096 after the fold)
SOFF = 1024.0     # per-core score re-centering (total 8192 = E||f||^2)
BIGDP = 1.0e6     # DP "no predecessor" value
RG = [list(range(NCORES))]
NT = M // 128     # 12 sample tiles

DEBUG = False     # add intermediate ExternalOutputs (test harness only)


def _flag(name, default="1"):
    return os.environ.get(name, default) == "1"


F_WARMUP = _flag("K_WARMUP", "0")  # tiny warmup collective first (net-zero)
F_RS = _flag("K_RS", "0")          # ReduceScatter scores (slower than A2A+fold)
# gather mode: 0 = 16 per-stripe gathers, 1 = 16KB-elem (2 gathers),
# 2 = 96-row batched (2 gathers + index replicate matmul)
F_BIGGATHER = int(os.environ.get("K_BIGGATHER", "2"))
F_SCAN = _flag("K_SCAN")           # tensor_tensor_scan DP (else antidiagonal)
F_XDMA = _flag("K_XDMA")           # x_all replicate via DMA (else PE matmuls)
F_TTR = _flag("K_TTR")             # per-stripe neg reduce (else monolithic)
F_GPS = _flag("K_GPS", "0")        # gpsimd for subtracts (3x slower than DVE)
F_SFIRST = _flag("K_SFIRST")       # scores A2A issued before centers A2A
# how many neg stripes use DVE mult+reduce instead of ACT square (balance)
N_DVE_NEG = int(os.environ.get("K_DVE_NEG", "3"))
# how many subtracts (of 16) go to GpSimd (concurrent GpSimd SBUF traffic
# slows DVE ops ~4x, so default none)
N_GPS_SUB = int(os.environ.get("K_GPS_SUB", "0"))


def build_body(tc, outs, ins):
    nc = tc.nc
    fnat = ins["fnat"]          # [M, DPC] natural feature slice (bf16)
    ftT = ins["ftT"]            # [DPC, M] pre-transposed slice (bf16)
    lf = ins["lf"]              # [M, D] local_features (stripe, d) bf16
    um = ins["um"]              # [33, 96] mask lhsT (bf16)
    vm = ins["vm"]              # [33, M] mask rhs (bf16)
    wsel = ins["wsel"]          # [64, NB] +I / -I rows (f32)
    ident = ins["ident"]        # [128, 128] identity (f32)
    esel = ins["esel"]          # [96, 96] grid-fold selection (bf16)
    aavg = ins["aavg"]          # [128, NT*N] packed chunk-avg matrix bf16
    fsel = ins["fsel"]          # [96, NB] fold selector (f32, fallback)
    repsel = ins["repsel"]      # [NB, 96] replicate selector (bf16, fallback)

    lf4 = lf.rearrange("m (j d) -> m j d", d=DL)   # [1536, 8, 1024]
    lf_v = lf.rearrange("m (j d) -> (m j) d", d=DL)

    sub_eng = nc.gpsimd if F_GPS else nc.vector

    with ExitStack() as ctx:
        const = ctx.enter_context(tc.tile_pool(name="const", bufs=1))
        pers = ctx.enter_context(tc.tile_pool(name="pers", bufs=1))
        dram = ctx.enter_context(tc.tile_pool(name="dram", bufs=1, space="DRAM"))

        # DRAM bounce buffers for collectives
        warm_in = dram.tile([NCORES, 1], F32)
        warm_out = dram.tile([NCORES, 1], F32)
        c2a_in = dram.tile([N, DPC], BF16)
        c2a_out = dram.tile([N, DPC], BF16)
        rs_in = dram.tile([N, M], BF16)
        rs_out = dram.tile([NB if F_RS else N, M], BF16)
        nrow_d = dram.tile([NT, 128], F32)

        # ---- warmup collective first: absorbs the initial cross-core
        # barrier + CC ring setup while the loads/compute run ----
        if F_WARMUP:
            nc.gpsimd.collective_compute(
                "AllToAll", ALU.bypass, replica_groups=RG,
                ins=[warm_in.opt()], outs=[warm_out.opt()],
            )

        # ---- big loads first so queues fill immediately ----
        nt_sb = [const.tile([128, DPC], BF16, name=f"nt{t}") for t in range(NT)]
        for t in range(NT):
            nc.sync.dma_start(nt_sb[t], fnat[t * 128:(t + 1) * 128, :])
        ftT_sb = [const.tile([128, M], BF16, name=f"ftT{kd}") for kd in range(8)]
        for kd in range(8):
            nc.sync.dma_start(ftT_sb[kd], ftT[kd * 128:(kd + 1) * 128, :])
        ident_sb = const.tile([128, 128], F32)
        nc.sync.dma_start(ident_sb, ident)
        wsel_sb = const.tile([64, NB], F32)
        nc.sync.dma_start(wsel_sb, wsel)
        esel_sb = const.tile([N, N], BF16)
        nc.sync.dma_start(esel_sb, esel)
        um_sb = const.tile([33, N], BF16)
        nc.sync.dma_start(um_sb, um)
        vm_sb = const.tile([33, M], BF16)
        nc.sync.dma_start(vm_sb, vm)
        if not F_RS:
            fsel_sb = const.tile([N, NB], BF16)
            nc.sync.dma_start(fsel_sb, fsel)
            nfsel_sb = const.tile([N, NB], BF16)
            nc.sync.dma_start(nfsel_sb, ins["nfsel"])
        if not F_XDMA:
            rep_sb = const.tile([NB, N], BF16)
            nc.sync.dma_start(rep_sb, repsel)
        if F_BIGGATHER == 2:
            rep8_sb = const.tile([NB, N], F32)
            nc.sync.dma_start(rep8_sb, ins["rep8"])
            joff_sb = const.tile([N, 1], F32)
            nc.sync.dma_start(joff_sb, ins["joff"])
            ins["rep8_sb"] = rep8_sb
            ins["joff_sb"] = joff_sb
        ones_c = const.tile([NB, 1], F32)
        nc.vector.memset(ones_c, 1.0)
        marg = const.tile([NB, 1], F32)
        nc.vector.memset(marg, MARGIN)
        idx0 = const.tile([2, 1], U32)
        nc.vector.memset(idx0, 0)
        a_sb = const.tile([128, NT * N], BF16)
        nc.sync.dma_start(a_sb, aavg)

        ctr_cm = pers.tile([N, DPC], F32)   # center-major centers slice (f32)
        ctr_bf = pers.tile([N, DPC], BF16)  # bf16 copy for the exchange
        x_all = pers.tile([N, D], BF16)     # centers replicated over stripes
        nrow = pers.tile([1, M], F32)       # partial sample norms row
        normcol = pers.tile([128, NT], F32)
        normT_sb = pers.tile([NT, 128], F32)

        # phase-2 grids
        if F_SCAN:
            dgrid = pers.tile([64, 64], F32)    # [b(+32 for neg), 8i x 8j]
            nc.vector.memset(dgrid, 0.0)
            bigrow = pers.tile([64, S], F32)
            nc.vector.memset(bigrow, BIGDP)
            dp_sb = pers.tile([64, 64], F32)    # DP rows
        else:
            dD = pers.tile([64, 81], F32)
            nc.vector.memset(dD, 0.0)
            dp = pers.tile([64, 81], F32)
            nc.vector.memset(dp, BIGDP)
            nc.vector.memset(dp[:, 1:2], 0.0)

        # ---- phase 1 ----
        with tc.tile_pool(name="psumc", bufs=1, space="PSUM") as psumc, \
             tc.tile_pool(name="psums", bufs=1, space="PSUM") as psums, \
             tc.tile_pool(name="psumt", bufs=2, space="PSUM") as psumt, \
             tc.tile_pool(name="workp1", bufs=2) as work:
            ps_c = [psumc.tile([N, 512], F32, name=f"ps_c{h}") for h in range(2)]
            s_ps = [psums.tile([N, 512], F32, name=f"s_ps{b}") for b in range(3)]
            for t in range(NT):
                for h in range(2):
                    nc.tensor.matmul(
                        ps_c[h], lhsT=a_sb[:, t * N:(t + 1) * N],
                        rhs=nt_sb[t][:, h * 512:(h + 1) * 512],
                        start=(t == 0), stop=(t == NT - 1),
                    )
                # partial sample norms: split ACT / DVE to halve the chain
                if F_TTR and t % 2 == 1:
                    sqv = work.tile([128, DPC], BF16, tag="sqv")
                    nc.vector.tensor_tensor(
                        out=sqv, in0=nt_sb[t], in1=nt_sb[t], op=ALU.mult
                    )
                    nc.vector.tensor_reduce(
                        normcol[:, t:t + 1], sqv,
                        axis=mybir.AxisListType.X, op=ALU.add,
                    )
                else:
                    sqd = work.tile([128, DPC], F32, tag="sqd")
                    nc.scalar.activation(
                        sqd, nt_sb[t], AF.Square, accum_out=normcol[:, t:t + 1]
                    )
            # centers psum -> f32 (DVE, for transposes) + bf16 (ACT, exchange)
            for h in range(2):
                nc.vector.tensor_copy(ctr_cm[:, h * 512:(h + 1) * 512], ps_c[h])
                nc.scalar.activation(
                    ctr_bf[:, h * 512:(h + 1) * 512], ps_c[h], AF.Copy
                )
            nc.sync.dma_start(c2a_in, ctr_bf)
            if not F_SFIRST:
                nc.gpsimd.collective_compute(
                    "AllToAll", ALU.bypass, replica_groups=RG,
                    ins=[c2a_in.opt()], outs=[c2a_out.opt()],
                )

            # mask k-tile first (only needs consts)
            for b in range(3):
                nc.tensor.matmul(
                    s_ps[b], lhsT=um_sb, rhs=vm_sb[:, b * 512:(b + 1) * 512],
                    start=True, stop=False,
                )
            # ctrT blocks scaled by -2 for the dot matmuls
            ctrT2 = []
            for kd in range(8):
                tpc = psumt.tile([128, N], F32, tag="tp")
                nc.tensor.transpose(
                    tpc, ctr_cm[:, kd * 128:(kd + 1) * 128], ident_sb[:N, :N]
                )
                c2 = work.tile([128, N], BF16, tag=f"ctrT{kd}", bufs=1)
                nc.vector.tensor_scalar_mul(c2, tpc, -2.0)
                ctrT2.append(c2)
            # norm row [1, 1536]: one transpose + DRAM-bounced rearrange
            ones_n = const.tile([1, N], F32)
            nc.vector.memset(ones_n, 1.0)
            tpn = psumt.tile([NT, 128], F32, tag="tpn", bufs=1)
            nc.tensor.transpose(tpn, normcol, ident_sb)
            nc.vector.tensor_copy(normT_sb, tpn)
            nc.sync.dma_start(nrow_d, normT_sb)
            nc.sync.dma_start(nrow, nrow_d.flatten().unsqueeze(0))
            # scores: accumulate -2 c.f over 8 k-tiles, then the norm row
            for kd in range(8):
                for b in range(3):
                    nc.tensor.matmul(
                        s_ps[b], lhsT=ctrT2[kd],
                        rhs=ftT_sb[kd][:, b * 512:(b + 1) * 512],
                        start=False, stop=False,
                    )
            for b in range(3):
                nc.tensor.matmul(
                    s_ps[b], lhsT=ones_n, rhs=nrow[:, b * 512:(b + 1) * 512],
                    start=False, stop=True,
                )
            # re-center by -SOFF so bf16 keeps mining precision, cast to bf16
            s_sb = pers.tile([N, M], BF16)
            for b in range(3):
                nc.vector.tensor_scalar(
                    out=s_sb[:, b * 512:(b + 1) * 512], in0=s_ps[b],
                    scalar1=-SOFF, scalar2=0.0, op0=ALU.add, op1=ALU.bypass,
                )
            nc.sync.dma_start(rs_in, s_sb)
            # dummy indirect gather: absorbs the SWDGE drain before the
            # real gathers land on the critical path
            dscr = pers.tile([2, 64], BF16)
            nc.gpsimd.indirect_dma_start(
                out=dscr, out_offset=None, in_=lf_v[:, :64],
                in_offset=bass.IndirectOffsetOnAxis(ap=idx0, axis=0),
            )
            if F_RS:
                cc_s = nc.gpsimd.collective_compute(
                    "ReduceScatter", ALU.add, replica_groups=RG,
                    ins=[rs_in.opt()], outs=[rs_out.opt()],
                )
            else:
                cc_s = nc.gpsimd.collective_compute(
                    "AllToAll", ALU.bypass, replica_groups=RG,
                    ins=[rs_in.opt()], outs=[rs_out.opt()],
                )
            if F_SFIRST:
                cc_c = nc.gpsimd.collective_compute(
                    "AllToAll", ALU.bypass, replica_groups=RG,
                    ins=[c2a_in.opt()], outs=[c2a_out.opt()],
                )
                # pin wire order: centers exchange strictly after scores
                bass._add_dep_helper(
                    cc_c.ins, cc_s.ins, sync=True,
                    reason="scores exchange gates the tail; centers second",
                )

        # preload the Sqrt activation table (slot beside Square) while the
        # collectives are on the wire, so phase 2 pays no sqrt table load
        sq_warm = pers.tile([1, S], F32)
        nc.vector.memset(sq_warm, 1.0)
        sq_warm2 = pers.tile([1, S], F32)
        nc.scalar.activation(sq_warm2, sq_warm, AF.Sqrt)

        # ---- x_all: replicate centers over stripe partitions ----
        # (runs while the scores exchange is on the wire)
        if F_XDMA:
            c2a3 = c2a_out.rearrange("(i b) d -> b i d", b=NB)   # [12, 8, 1024]
            x4 = x_all.rearrange("(b j) (i d) -> b j i d", j=S, d=DL)
            for j in range(S):
                nc.sync.dma_start(x4[:, j], c2a3)
        else:
            xc3 = pers.tile([NB, S, DL], BF16)
            nc.sync.dma_start(xc3, c2a_out.rearrange("(i b) d -> b i d", b=NB))
            with tc.tile_pool(name="psumx", bufs=4, space="PSUM") as psumx:
                for i in range(S):
                    for h in range(2):
                        xr = psumx.tile([N, 512], F32, tag="xr")
                        nc.tensor.matmul(
                            xr, lhsT=rep_sb,
                            rhs=xc3[:, i, h * 512:(h + 1) * 512],
                            start=True, stop=True,
                        )
                        lo = i * DL + h * 512
                        if i % 2 == 0:
                            nc.vector.tensor_copy(x_all[:, lo:lo + 512], xr)
                        else:
                            nc.scalar.activation(x_all[:, lo:lo + 512], xr, AF.Copy)

        # ---- mining on this core's 12 rows ----
        pmax = pers.tile([NB, 8], F32)
        pidx = pers.tile([NB, 8], U32)
        nmax = pers.tile([NB, 8], F32)
        nidx = pers.tile([NB, 8], U32)
        s12 = pers.tile([NB, M], F32, name="s12") if DEBUG else None
        mine_ctx = ExitStack()
        if F_RS:
            s12b = pers.tile([NB, M], BF16)
            nc.sync.dma_start(s12b, rs_out)
            nneg = pers.tile([NB, M], BF16)
            nc.vector.tensor_scalar_mul(nneg, s12b, -1.0)
            nc.vector.max(pmax, s12b)
            nc.vector.max_index(pidx, pmax, s12b)
            nc.vector.max(nmax, nneg)
            nc.vector.max_index(nidx, nmax, nneg)
            if DEBUG:
                nc.vector.tensor_copy(s12, s12b)
        else:
            # dual fold: +fsel and -fsel matmuls give the positive-mining
            # matrix and its negation straight in PSUM (no DVE negate)
            s8 = pers.tile([N, M], BF16, name="s8")
            for b in range(3):
                nc.sync.dma_start(
                    s8[:, b * 512:(b + 1) * 512],
                    rs_out[:, b * 512:(b + 1) * 512],
                )
            psumm = mine_ctx.enter_context(
                tc.tile_pool(name="psumm", bufs=1, space="PSUM"))
            pm = psumm.tile([NB, M], F32)
            nm = psumm.tile([NB, M], F32)
            for b in range(3):
                nc.tensor.matmul(
                    pm[:, b * 512:(b + 1) * 512], lhsT=fsel_sb,
                    rhs=s8[:, b * 512:(b + 1) * 512], start=True, stop=True,
                )
                nc.tensor.matmul(
                    nm[:, b * 512:(b + 1) * 512], lhsT=nfsel_sb,
                    rhs=s8[:, b * 512:(b + 1) * 512], start=True, stop=True,
                )
            nc.vector.max(pmax, pm)
            nc.vector.max_index(pidx, pmax, pm)
            nc.vector.max(nmax, nm)
            nc.vector.max_index(nidx, nmax, nm)
            if DEBUG:
                nc.vector.tensor_copy(s12, pm)

        # ---- phase 2: winner-row gathers + local distances ----
        y96p = pers.tile([N, DL], BF16)
        y96n = pers.tile([N, DL], BF16)
        if F_BIGGATHER == 1:
            nc.gpsimd.indirect_dma_start(
                out=y96p.rearrange("(b j) d -> b j d", j=S), out_offset=None,
                in_=lf4,
                in_offset=bass.IndirectOffsetOnAxis(ap=pidx[:, 0:1], axis=0),
            )
            nc.gpsimd.indirect_dma_start(
                out=y96n.rearrange("(b j) d -> b j d", j=S), out_offset=None,
                in_=lf4,
                in_offset=bass.IndirectOffsetOnAxis(ap=nidx[:, 0:1], axis=0),
            )
        elif F_BIGGATHER == 2:
            # replicate each winner index to its 8 stripe partitions via a
            # tiny f32 matmul (entries 8.0 -> 8*idx, exact for idx < 2^19),
            # add the per-partition stripe offset, one 96-row gather per set
            rep8_sb = ins["rep8_sb"]
            joff_sb = ins["joff_sb"]
            with tc.tile_pool(name="psumg", bufs=1, space="PSUM") as psumg:
                for iset, (idx, y96) in enumerate(((pidx, y96p), (nidx, y96n))):
                    idf = pers.tile([NB, 1], F32, tag="idf", bufs=2)
                    nc.vector.tensor_copy(idf, idx[:, 0:1])
                    rp = psumg.tile([N, 1], F32, tag="rp", bufs=2)
                    nc.tensor.matmul(rp, lhsT=rep8_sb, rhs=idf,
                                     start=True, stop=True)
                    pj96 = pers.tile([N, 1], U32, tag="pj96", bufs=2)
                    nc.vector.tensor_tensor(
                        out=pj96, in0=rp, in1=joff_sb, op=ALU.add
                    )
                    nc.gpsimd.indirect_dma_start(
                        out=y96, out_offset=None, in_=lf_v,
                        in_offset=bass.IndirectOffsetOnAxis(ap=pj96, axis=0),
                    )
        else:
            y3p = y96p.rearrange("(b j) d -> b j d", j=S)
            y3n = y96n.rearrange("(b j) d -> b j d", j=S)
            for iset, (idx, y3) in enumerate(((pidx, y3p), (nidx, y3n))):
                for j in range(S):
                    pj = pers.tile([NB, 1], U32, tag="pj", bufs=4)
                    nc.vector.tensor_scalar(
                        out=pj, in0=idx[:, :1], scalar1=8, scalar2=j,
                        op0=ALU.mult, op1=ALU.add,
                    )
                    nc.gpsimd.indirect_dma_start(
                        out=y3[:, j, :], out_offset=None, in_=lf_v,
                        in_offset=bass.IndirectOffsetOnAxis(ap=pj, axis=0),
                    )
        mine_ctx.close()

        d2t = pers.tile([N, 2 * S], F32)   # cols 0-7 pos, 8-15 neg
        with tc.tile_pool(name="workp2", bufs=3) as work2:
            # positive diffs first so ACT can chew squares continuously
            dtp = []
            for i in range(S):
                xsl = x_all[:, i * DL:(i + 1) * DL]
                dt = work2.tile([N, DL], BF16, tag=f"dtp{i}", bufs=1)
                nc.vector.tensor_tensor(out=dt, in0=y96p, in1=xsl,
                                        op=ALU.subtract)
                dtp.append(dt)
            for i in range(S):
                sqp = work2.tile([N, DL], BF16, tag="sqp")
                nc.scalar.activation(
                    sqp, dtp[i], AF.Square, accum_out=d2t[:, i:i + 1]
                )
            # negatives: subtracts split DVE/GpSimd, square-sums split ACT/DVE
            for i in range(S):
                xsl = x_all[:, i * DL:(i + 1) * DL]
                dtn = work2.tile([N, DL], BF16, tag="dtn")
                eng = nc.gpsimd if i < N_GPS_SUB else nc.vector
                eng.tensor_tensor(out=dtn, in0=y96n, in1=xsl, op=ALU.subtract)
                if i < S - N_DVE_NEG:
                    sqn = work2.tile([N, DL], BF16, tag="sqn")
                    nc.scalar.activation(
                        sqn, dtn, AF.Square, accum_out=d2t[:, S + i:S + i + 1]
                    )
                else:
                    sqn = work2.tile([N, DL], BF16, tag="sqn")
                    nc.vector.tensor_tensor(
                        out=sqn, in0=dtn, in1=dtn, op=ALU.mult
                    )
                    nc.vector.tensor_reduce(
                        d2t[:, S + i:S + i + 1], sqn,
                        axis=mybir.AxisListType.X, op=ALU.add,
                    )

            dcl = work2.tile([N, 2 * S], F32, tag="dcl", bufs=1)
            nc.vector.tensor_scalar_max(dcl, d2t, 1e-12)
            dsqr = work2.tile([N, 2 * S], F32, tag="dsqr", bufs=1)
            nc.scalar.activation(dsqr, dcl, AF.Sqrt)
            dsg = work2.tile([N, 2 * S], BF16, tag="dsg", bufs=1)
            nc.scalar.activation(dsg, dsqr, AF.Tanh, scale=0.5)
            # fold [96 (b,j), 16 (iset,i)] -> DP grid layout
            if F_SCAN:
                with tc.tile_pool(name="psf", bufs=1, space="PSUM") as psf_p:
                    psf = psf_p.tile([NB, 8 * 16], F32)
                    for j in range(S):
                        nc.tensor.matmul(
                            psf[:, j * 16:(j + 1) * 16],
                            lhsT=esel_sb[:, j * NB:(j + 1) * NB],
                            rhs=dsg, start=True, stop=True,
                        )
                    psf_v = psf.rearrange("p (j c) -> p c j", c=16)  # [12,16,8]
                    dg_v = dgrid.rearrange("p (i j) -> p i j", j=S)
                    nc.vector.tensor_copy(dg_v[0:NB], psf_v[:, 0:S, :])
                    nc.vector.tensor_copy(dg_v[32:32 + NB], psf_v[:, S:16, :])
            else:
                for iset in range(2):
                    with tc.tile_pool(name=f"psf{iset}", bufs=1,
                                      space="PSUM") as psf_p:
                        psf = psf_p.tile([NB, 64], F32)
                        for j in range(S):
                            nc.tensor.matmul(
                                psf[:, j * S:(j + 1) * S],
                                lhsT=esel_sb[:, j * NB:(j + 1) * NB],
                                rhs=dsg[:, iset * S:(iset + 1) * S],
                                start=True, stop=True,
                            )
                        dst = dD[iset * 32:iset * 32 + NB, 0:72].rearrange(
                            "p (i j) -> p j i", j=9
                        )[:, 1:9, :]
                        nc.vector.tensor_copy(
                            dst, psf.rearrange("p (j i) -> p j i", i=S)
                        )

        # ---- shortest-path DP ----
        if F_SCAN:
            for i in range(S):
                prev = bigrow if i == 0 else dp_sb[:, (i - 1) * S:i * S]
                nc.vector.tensor_tensor_scan(
                    out=dp_sb[:, i * S:(i + 1) * S],
                    data0=prev,
                    data1=dgrid[:, i * S:(i + 1) * S],
                    initial=0.0 if i == 0 else BIGDP,
                    op0=ALU.min, op1=ALU.add,
                )
            dp_last = dp_sb[:, 63:64]
        else:
            for kdiag in range(2, 17):
                lo = max(1, kdiag - 8)
                hi = min(8, kdiag - 1)
                cnt = hi - lo + 1
                f0 = 9 * lo + (kdiag - lo)
                t = pers.tile([64, 8], F32, tag="dptmp", bufs=2)
                nc.vector.tensor_tensor(
                    out=t[:, :cnt],
                    in0=dp[:, f0 - 9:f0 - 9 + 8 * (cnt - 1) + 1:8],
                    in1=dp[:, f0 - 1:f0 - 1 + 8 * (cnt - 1) + 1:8],
                    op=ALU.min,
                )
                nc.vector.tensor_tensor(
                    out=dp[:, f0:f0 + 8 * (cnt - 1) + 1:8],
                    in0=t[:, :cnt],
                    in1=dD[:, f0 - 9:f0 - 9 + 8 * (cnt - 1) + 1:8],
                    op=ALU.add,
                )
            dp_last = dp[:, 80:81]

        # ---- loss: relu(ap - an + margin), partial sum over 12 centers ----
        with tc.tile_pool(name="psum3", bufs=1, space="PSUM") as psum3:
            dps = psum3.tile([NB, 1], F32)
            nc.tensor.matmul(dps, lhsT=wsel_sb, rhs=dp_last, start=True, stop=True)
            r12 = pers.tile([NB, 1], F32)
            nc.vector.tensor_scalar(
                out=r12, in0=dps, scalar1=marg, scalar2=0.0,
                op0=ALU.add, op1=ALU.max,
            )
            lsum = psum3.tile([1, 1], F32)
            nc.tensor.matmul(lsum, lhsT=r12, rhs=ones_c, start=True, stop=True)
            out_sb = pers.tile([1, 1], F32)
            nc.vector.tensor_copy(out_sb, lsum)
        nc.sync.dma_start(outs["out"], out_sb)

        if DEBUG:
            nc.sync.dma_start(outs["dbg_pidx"], pidx)
            nc.sync.dma_start(outs["dbg_nidx"], nidx)
            nc.sync.dma_start(outs["dbg_d2t"], d2t)
            if F_SCAN:
                nc.sync.dma_start(outs["dbg_dgrid"], dgrid)
                nc.sync.dma_start(outs["dbg_dp"], dp_sb)
            nc.sync.dma_start(outs["dbg_s12"], s12)


def build_program():
    nc = bacc.Bacc(
        "TRN2", target_bir_lowering=False, debug=False,
        enable_asserts=False, num_devices=NCORES,
    )
    ins = {
        "fnat": nc.dram_tensor("fnat", [M, DPC], BF16, kind="ExternalInput").ap(),
        "ftT": nc.dram_tensor("ftT", [DPC, M], BF16, kind="ExternalInput").ap(),
        "lf": nc.dram_tensor("lf", [M, D], BF16, kind="ExternalInput").ap(),
        "um": nc.dram_tensor("um", [33, N], BF16, kind="ExternalInput").ap(),
        "vm": nc.dram_tensor("vm", [33, M], BF16, kind="ExternalInput").ap(),
        "wsel": nc.dram_tensor("wsel", [64, NB], F32, kind="ExternalInput").ap(),
        "ident": nc.dram_tensor("ident", [128, 128], F32, kind="ExternalInput").ap(),
        "esel": nc.dram_tensor("esel", [N, N], BF16, kind="ExternalInput").ap(),
        "aavg": nc.dram_tensor("aavg", [128, NT * N], BF16, kind="ExternalInput").ap(),
        "fsel": nc.dram_tensor("fsel", [N, NB], BF16, kind="ExternalInput").ap(),
        "repsel": nc.dram_tensor("repsel", [NB, N], BF16, kind="ExternalInput").ap(),
        "rep8": nc.dram_tensor("rep8", [NB, N], F32, kind="ExternalInput").ap(),
        "joff": nc.dram_tensor("joff", [N, 1], F32, kind="ExternalInput").ap(),
        "nfsel": nc.dram_tensor("nfsel", [N, NB], BF16, kind="ExternalInput").ap(),
    }
    outs = {"out": nc.dram_tensor("out", [1, 1], F32, kind="ExternalOutput").ap()}
    if DEBUG:
        outs["dbg_pidx"] = nc.dram_tensor(
            "dbg_pidx", [NB, 8], U32, kind="ExternalOutput").ap()
        outs["dbg_nidx"] = nc.dram_tensor(
            "dbg_nidx", [NB, 8], U32, kind="ExternalOutput").ap()
        outs["dbg_d2t"] = nc.dram_tensor(
            "dbg_d2t", [N, 2 * S], F32, kind="ExternalOutput").ap()
        if F_SCAN:
            outs["dbg_dgrid"] = nc.dram_tensor(
                "dbg_dgrid", [64, 64], F32, kind="ExternalOutput").ap()
            outs["dbg_dp"] = nc.dram_tensor(
                "dbg_dp", [64, 64], F32, kind="ExternalOutput").ap()
        outs["dbg_s12"] = nc.dram_tensor(
            "dbg_s12", [NB, M], F32, kind="ExternalOutput").ap()
    with tile.TileContext(nc) as tc:
        build_body(tc, outs, ins)
    nc.compile()
    return nc


def make_in_maps(feats, labels, local_features):
    bf16 = mybir.dt.np(BF16)
    feats = np.asarray(feats, dtype=np.float32).astype(bf16)
    labels = np.asarray(labels).astype(np.int64)
    lf_flat = np.ascontiguousarray(
        np.asarray(local_features, dtype=np.float32)
        .transpose(0, 2, 1).reshape(M, D)
    ).astype(bf16)
    anchors = labels[::K]  # [96]
    # mask factors: sum_r um[r,n]*vm[r,m] = -BIGM * (anchors[n] != labels[m])
    um = np.zeros((33, N), dtype=np.float32)
    vm = np.zeros((33, M), dtype=np.float32)
    um[0, :] = -BIGM
    vm[0, :] = 1.0
    for ell in range(L):
        um[1 + ell, :] = BIGM * (anchors == ell)
        vm[1 + ell, :] = (labels == ell)
    um = um.astype(bf16)
    vm = vm.astype(bf16)
    # fold selector: exchange-out row (NB*i + b) -> out row b
    fsel = np.zeros((N, NB), dtype=np.float32)
    for i in range(NCORES):
        for b in range(NB):
            fsel[NB * i + b, b] = 1.0
    fsel_b = fsel.astype(bf16)
    nfsel_b = (-fsel).astype(bf16)
    # replicate selector: center b -> partitions b*8..b*8+8 (fallback)
    repsel = np.zeros((NB, N), dtype=np.float32)
    for b in range(NB):
        repsel[b, b * S:(b + 1) * S] = 1.0
    repsel = repsel.astype(bf16)
    wsel = np.zeros((64, NB), dtype=np.float32)
    wsel[0:NB, :] = np.eye(NB)
    wsel[32:32 + NB, :] = -np.eye(NB)
    ident = np.eye(128, dtype=np.float32)
    esel = np.zeros((N, N), dtype=np.float32)
    for b in range(NB):
        for j in range(S):
            esel[b * S + j, j * NB + b] = 1.0
    esel = esel.astype(bf16)
    # packed chunk-averaging matrix: [128, t*96+n]
    aavg = np.zeros((M, N), dtype=np.float32)
    aavg[np.arange(M), np.arange(M) // K] = 1.0 / K
    aavg_p = np.ascontiguousarray(
        aavg.reshape(NT, 128, N).transpose(1, 0, 2).reshape(128, NT * N)
    ).astype(bf16)
    in_maps = []
    for c in range(NCORES):
        sl = feats[:, c * DPC:(c + 1) * DPC]
        in_maps.append({
            "fnat": np.ascontiguousarray(sl),
            "ftT": np.ascontiguousarray(sl.T),
            "lf": lf_flat,
            "um": um,
            "vm": vm,
            "wsel": wsel,
            "ident": ident,
            "esel": esel,
            "aavg": aavg_p,
            "fsel": fsel_b,
            "repsel": repsel,
            "rep8": 8.0 * repsel.astype(np.float32),
            "joff": np.tile(np.arange(S, dtype=np.float32), NB)[:, None],
            "nfsel": nfsel_b,
        })
    return in_maps


_NC_CACHE = None


def _get_nc():
    global _NC_CACHE
    if _NC_CACHE is None:
        _NC_CACHE = build_program()
    return _NC_CACHE


def run(feats, labels, local_features, trace=False, **kwargs):
    nc = _get_nc()
    in_maps = make_in_maps(feats, labels, local_features)
    res = bass_utils.run_bass_kernel_spmd(
        nc, in_maps, core_ids=list(range(NCORES)), trace=trace, **kwargs
    )
    partial = sum(float(r["out"][0, 0]) for r in res.results)
    return np.float32(partial / N), res


def kernel(feats, labels, local_features):
    loss, _ = run(feats, labels, local_features)
    return loss


# revision 39
# speedup vs baseline: 1.8040x; 1.0938x over previous
"""Bass/Tile TRN2 kernel for nn_CenterAlignedTripletLoss (8-core SPMD).

Sharding (feature-parallel):
  Phase 1: feats sharded along the feature axis (8192 -> 1024 per core).
    Each core computes partial mining scores for all 96 centers over its
    slice:  s[n, m] = ||f_m||^2 - 2 c_n . f_m - BIGM*(1-is_pos[n,m])
    accumulated in PSUM (the label mask enters as a rank-33 matmul
    k-tile; partial norms as a rank-1 f32 k-tile; the row-constant
    ||c_n||^2 is omitted).  Norm tiles split ACT/DVE (tensor_tensor_
    reduce).  A tiny warmup AllToAll issued first absorbs the initial
    cross-core barrier + CC setup into the load phase.
  Exchange: Mesh AllToAll of bf16 center slices (each core gets
    full-feature centers for its 12-center block), then ReduceScatter
    (add) of the f32 partial scores so each core receives its 12 rows
    already summed -- no fold matmul.
  Mining: hardest positive = row argmax, hardest negative = row argmax
    of the negated matrix.
  Phase 2: one indirect-DMA gather per set (12 descriptors x 16KB: the
    winner's full 8-stripe local row, landing as (center,stripe)
    partition layout), stripe distances via GpSimd subtract + ACT
    square-accumulate (positive) / DVE tensor_tensor_reduce (negative),
    sqrt -> tanh(d/2) (sqrt table preloaded off-path), PE fold to the
    DP grid, then the AlignedReID shortest path as 8 row-wise
    tensor_tensor_scan(min, add) ops.  Partial loss summed on host.
"""

import os
import numpy as np
from contextlib import ExitStack

import concourse.bass as bass
import concourse.bacc as bacc
import concourse.tile as tile
from concourse import mybir
from concourse import bass_utils

F32 = mybir.dt.float32
BF16 = mybir.dt.bfloat16
U32 = mybir.dt.uint32
AF = mybir.ActivationFunctionType
ALU = mybir.AluOpType

NCORES = 8
M = 1536          # samples
D = 8192          # feature dim
DPC = D // NCORES # 1024 features per core
N = 96            # centers
NB = N // NCORES  # 12 centers per core
K = 16            # samples per chunk
S = 8             # stripes
DL = 1024         # local feature dim per stripe
L = 32            # label count
MARGIN = 0.3
BIGM = 512.0      # per-core mask magnitude (sums to 4096 after the fold)
SOFF = 1024.0     # per-core score re-centering (total 8192 = E||f||^2)
BIGDP = 1.0e6     # DP "no predecessor" value
RG = [list(range(NCORES))]
NT = M // 128     # 12 sample tiles

DEBUG = False     # add intermediate ExternalOutputs (test harness only)


def _flag(name, default="1"):
    return os.environ.get(name, default) == "1"


F_WARMUP = _flag("K_WARMUP", "0")  # tiny warmup collective first (net-zero)
F_RS = _flag("K_RS", "0")          # ReduceScatter scores (slower than A2A+fold)
# gather mode: 0 = 16 per-stripe gathers, 1 = 16KB-elem (2 gathers),
# 2 = 96-row batched (2 gathers + index replicate matmul)
F_BIGGATHER = int(os.environ.get("K_BIGGATHER", "2"))
F_SCAN = _flag("K_SCAN")           # tensor_tensor_scan DP (else antidiagonal)
# x_all replicate: DMA fill measured ~14.6us post-centers-A2A (gates the
# distance block); PE matmuls overlap the scores A2A instead -> default off
F_XDMA = _flag("K_XDMA", "0")
F_TTR = _flag("K_TTR")             # per-stripe neg reduce (else monolithic)
F_GPS = _flag("K_GPS", "0")        # gpsimd for subtracts (3x slower than DVE)
# scores A2A first delays centers+x_all past the gather, stalling the
# distance block ~9us; centers-first overlaps x_all with the scores wire
F_SFIRST = _flag("K_SFIRST", "0")
# how many neg stripes use DVE mult+reduce instead of ACT square (balance)
N_DVE_NEG = int(os.environ.get("K_DVE_NEG", "3"))
# how many subtracts (of 16) go to GpSimd (concurrent GpSimd SBUF traffic
# slows DVE ops ~4x, so default none)
N_GPS_SUB = int(os.environ.get("K_GPS_SUB", "0"))


def build_body(tc, outs, ins):
    nc = tc.nc
    fnat = ins["fnat"]          # [M, DPC] natural feature slice (bf16)
    ftT = ins["ftT"]            # [DPC, M] pre-transposed slice (bf16)
    lf = ins["lf"]              # [M, D] local_features (stripe, d) bf16
    um = ins["um"]              # [33, 96] mask lhsT (bf16)
    vm = ins["vm"]              # [33, M] mask rhs (bf16)
    wsel = ins["wsel"]          # [64, NB] +I / -I rows (f32)
    ident = ins["ident"]        # [128, 128] identity (f32)
    esel = ins["esel"]          # [96, 96] grid-fold selection (bf16)
    aavg = ins["aavg"]          # [128, NT*N] packed chunk-avg matrix bf16
    fsel = ins["fsel"]          # [96, NB] fold selector (f32, fallback)
    repsel = ins["repsel"]      # [NB, 96] replicate selector (bf16, fallback)

    lf4 = lf.rearrange("m (j d) -> m j d", d=DL)   # [1536, 8, 1024]
    lf_v = lf.rearrange("m (j d) -> (m j) d", d=DL)

    sub_eng = nc.gpsimd if F_GPS else nc.vector

    with ExitStack() as ctx:
        const = ctx.enter_context(tc.tile_pool(name="const", bufs=1))
        pers = ctx.enter_context(tc.tile_pool(name="pers", bufs=1))
        dram = ctx.enter_context(tc.tile_pool(name="dram", bufs=1, space="DRAM"))

        # DRAM bounce buffers for collectives
        warm_in = dram.tile([NCORES, 1], F32)
        warm_out = dram.tile([NCORES, 1], F32)
        c2a_in = dram.tile([N, DPC], BF16)
        c2a_out = dram.tile([N, DPC], BF16)
        rs_in = dram.tile([N, M], BF16)
        rs_out = dram.tile([NB if F_RS else N, M], BF16)
        nrow_d = dram.tile([NT, 128], F32)

        # ---- warmup collective first: absorbs the initial cross-core
        # barrier + CC ring setup while the loads/compute run ----
        if F_WARMUP:
            nc.gpsimd.collective_compute(
                "AllToAll", ALU.bypass, replica_groups=RG,
                ins=[warm_in.opt()], outs=[warm_out.opt()],
            )

        # ---- big loads first so queues fill immediately ----
        nt_sb = [const.tile([128, DPC], BF16, name=f"nt{t}") for t in range(NT)]
        for t in range(NT):
            nc.sync.dma_start(nt_sb[t], fnat[t * 128:(t + 1) * 128, :])
        ftT_sb = [const.tile([128, M], BF16, name=f"ftT{kd}") for kd in range(8)]
        for kd in range(8):
            nc.sync.dma_start(ftT_sb[kd], ftT[kd * 128:(kd + 1) * 128, :])
        ident_sb = const.tile([128, 128], F32)
        nc.sync.dma_start(ident_sb, ident)
        wsel_sb = const.tile([64, NB], F32)
        nc.sync.dma_start(wsel_sb, wsel)
        esel_sb = const.tile([N, N], BF16)
        nc.sync.dma_start(esel_sb, esel)
        um_sb = const.tile([33, N], BF16)
        nc.sync.dma_start(um_sb, um)
        vm_sb = const.tile([33, M], BF16)
        nc.sync.dma_start(vm_sb, vm)
        if not F_RS:
            fsel_sb = const.tile([N, NB], BF16)
            nc.sync.dma_start(fsel_sb, fsel)
            nfsel_sb = const.tile([N, NB], BF16)
            nc.sync.dma_start(nfsel_sb, ins["nfsel"])
        if not F_XDMA:
            rep_sb = const.tile([NB, N], BF16)
            nc.sync.dma_start(rep_sb, repsel)
        if F_BIGGATHER == 2:
            rep8_sb = const.tile([NB, N], F32)
            nc.sync.dma_start(rep8_sb, ins["rep8"])
            joff_sb = const.tile([N, 1], F32)
            nc.sync.dma_start(joff_sb, ins["joff"])
            ins["rep8_sb"] = rep8_sb
            ins["joff_sb"] = joff_sb
        ones_c = const.tile([NB, 1], F32)
        nc.vector.memset(ones_c, 1.0)
        marg = const.tile([NB, 1], F32)
        nc.vector.memset(marg, MARGIN)
        idx0 = const.tile([2, 1], U32)
        nc.vector.memset(idx0, 0)
        a_sb = const.tile([128, NT * N], BF16)
        nc.sync.dma_start(a_sb, aavg)

        ctr_cm = pers.tile([N, DPC], F32)   # center-major centers slice (f32)
        ctr_bf = pers.tile([N, DPC], BF16)  # bf16 copy for the exchange
        x_all = pers.tile([N, D], BF16)     # centers replicated over stripes
        nrow = pers.tile([1, M], F32)       # partial sample norms row
        normcol = pers.tile([128, NT], F32)
        normT_sb = pers.tile([NT, 128], F32)

        # phase-2 grids
        if F_SCAN:
            dgrid = pers.tile([64, 64], F32)    # [b(+32 for neg), 8i x 8j]
            nc.vector.memset(dgrid, 0.0)
            bigrow = pers.tile([64, S], F32)
            nc.vector.memset(bigrow, BIGDP)
            dp_sb = pers.tile([64, 64], F32)    # DP rows
        else:
            dD = pers.tile([64, 81], F32)
            nc.vector.memset(dD, 0.0)
            dp = pers.tile([64, 81], F32)
            nc.vector.memset(dp, BIGDP)
            nc.vector.memset(dp[:, 1:2], 0.0)

        # ---- phase 1 ----
        with tc.tile_pool(name="psumc", bufs=1, space="PSUM") as psumc, \
             tc.tile_pool(name="psums", bufs=1, space="PSUM") as psums, \
             tc.tile_pool(name="psumt", bufs=2, space="PSUM") as psumt, \
             tc.tile_pool(name="workp1", bufs=2) as work:
            ps_c = [psumc.tile([N, 512], F32, name=f"ps_c{h}") for h in range(2)]
            s_ps = [psums.tile([N, 512], F32, name=f"s_ps{b}") for b in range(3)]
            for t in range(NT):
                for h in range(2):
                    nc.tensor.matmul(
                        ps_c[h], lhsT=a_sb[:, t * N:(t + 1) * N],
                        rhs=nt_sb[t][:, h * 512:(h + 1) * 512],
                        start=(t == 0), stop=(t == NT - 1),
                    )
                # partial sample norms: split ACT / DVE to halve the chain
                if F_TTR and t % 2 == 1:
                    sqv = work.tile([128, DPC], BF16, tag="sqv")
                    nc.vector.tensor_tensor(
                        out=sqv, in0=nt_sb[t], in1=nt_sb[t], op=ALU.mult
                    )
                    nc.vector.tensor_reduce(
                        normcol[:, t:t + 1], sqv,
                        axis=mybir.AxisListType.X, op=ALU.add,
                    )
                else:
                    sqd = work.tile([128, DPC], F32, tag="sqd")
                    nc.scalar.activation(
                        sqd, nt_sb[t], AF.Square, accum_out=normcol[:, t:t + 1]
                    )
            # centers psum -> f32 (DVE, for transposes) + bf16 (ACT, exchange)
            for h in range(2):
                nc.vector.tensor_copy(ctr_cm[:, h * 512:(h + 1) * 512], ps_c[h])
                nc.scalar.activation(
                    ctr_bf[:, h * 512:(h + 1) * 512], ps_c[h], AF.Copy
                )
            nc.sync.dma_start(c2a_in, ctr_bf)
            if not F_SFIRST:
                nc.gpsimd.collective_compute(
                    "AllToAll", ALU.bypass, replica_groups=RG,
                    ins=[c2a_in.opt()], outs=[c2a_out.opt()],
                )

            # mask k-tile first (only needs consts)
            for b in range(3):
                nc.tensor.matmul(
                    s_ps[b], lhsT=um_sb, rhs=vm_sb[:, b * 512:(b + 1) * 512],
                    start=True, stop=False,
                )
            # ctrT blocks scaled by -2 for the dot matmuls
            ctrT2 = []
            for kd in range(8):
                tpc = psumt.tile([128, N], F32, tag="tp")
                nc.tensor.transpose(
                    tpc, ctr_cm[:, kd * 128:(kd + 1) * 128], ident_sb[:N, :N]
                )
                c2 = work.tile([128, N], BF16, tag=f"ctrT{kd}", bufs=1)
                nc.vector.tensor_scalar_mul(c2, tpc, -2.0)
                ctrT2.append(c2)
            # norm row [1, 1536]: one transpose + DRAM-bounced rearrange
            ones_n = const.tile([1, N], F32)
            nc.vector.memset(ones_n, 1.0)
            tpn = psumt.tile([NT, 128], F32, tag="tpn", bufs=1)
            nc.tensor.transpose(tpn, normcol, ident_sb)
            nc.vector.tensor_copy(normT_sb, tpn)
            nc.sync.dma_start(nrow_d, normT_sb)
            nc.sync.dma_start(nrow, nrow_d.flatten().unsqueeze(0))
            # scores: accumulate -2 c.f over 8 k-tiles, then the norm row
            for kd in range(8):
                for b in range(3):
                    nc.tensor.matmul(
                        s_ps[b], lhsT=ctrT2[kd],
                        rhs=ftT_sb[kd][:, b * 512:(b + 1) * 512],
                        start=False, stop=False,
                    )
            for b in range(3):
                nc.tensor.matmul(
                    s_ps[b], lhsT=ones_n, rhs=nrow[:, b * 512:(b + 1) * 512],
                    start=False, stop=True,
                )
            # re-center by -SOFF so bf16 keeps mining precision, cast to bf16
            s_sb = pers.tile([N, M], BF16)
            for b in range(3):
                nc.vector.tensor_scalar(
                    out=s_sb[:, b * 512:(b + 1) * 512], in0=s_ps[b],
                    scalar1=-SOFF, scalar2=0.0, op0=ALU.add, op1=ALU.bypass,
                )
            nc.sync.dma_start(rs_in, s_sb)
            # dummy indirect gather: absorbs the SWDGE drain before the
            # real gathers land on the critical path
            dscr = pers.tile([2, 64], BF16)
            nc.gpsimd.indirect_dma_start(
                out=dscr, out_offset=None, in_=lf_v[:, :64],
                in_offset=bass.IndirectOffsetOnAxis(ap=idx0, axis=0),
            )
            if F_RS:
                cc_s = nc.gpsimd.collective_compute(
                    "ReduceScatter", ALU.add, replica_groups=RG,
                    ins=[rs_in.opt()], outs=[rs_out.opt()],
                )
            else:
                cc_s = nc.gpsimd.collective_compute(
                    "AllToAll", ALU.bypass, replica_groups=RG,
                    ins=[rs_in.opt()], outs=[rs_out.opt()],
                )
            if F_SFIRST:
                cc_c = nc.gpsimd.collective_compute(
                    "AllToAll", ALU.bypass, replica_groups=RG,
                    ins=[c2a_in.opt()], outs=[c2a_out.opt()],
                )
                # pin wire order: centers exchange strictly after scores
                bass._add_dep_helper(
                    cc_c.ins, cc_s.ins, sync=True,
                    reason="scores exchange gates the tail; centers second",
                )

        # preload the Sqrt activation table (slot beside Square) while the
        # collectives are on the wire, so phase 2 pays no sqrt table load
        sq_warm = pers.tile([1, S], F32)
        nc.vector.memset(sq_warm, 1.0)
        sq_warm2 = pers.tile([1, S], F32)
        nc.scalar.activation(sq_warm2, sq_warm, AF.Sqrt)

        # ---- x_all: replicate centers over stripe partitions ----
        # (runs while the scores exchange is on the wire)
        if F_XDMA:
            c2a3 = c2a_out.rearrange("(i b) d -> b i d", b=NB)   # [12, 8, 1024]
            x4 = x_all.rearrange("(b j) (i d) -> b j i d", j=S, d=DL)
            for j in range(S):
                nc.sync.dma_start(x4[:, j], c2a3)
        else:
            xc3 = pers.tile([NB, S, DL], BF16)
            nc.sync.dma_start(xc3, c2a_out.rearrange("(i b) d -> b i d", b=NB))
            with tc.tile_pool(name="psumx", bufs=4, space="PSUM") as psumx:
                for i in range(S):
                    for h in range(2):
                        xr = psumx.tile([N, 512], F32, tag="xr")
                        nc.tensor.matmul(
                            xr, lhsT=rep_sb,
                            rhs=xc3[:, i, h * 512:(h + 1) * 512],
                            start=True, stop=True,
                        )
                        lo = i * DL + h * 512
                        if i % 2 == 0:
                            nc.vector.tensor_copy(x_all[:, lo:lo + 512], xr)
                        else:
                            nc.scalar.activation(x_all[:, lo:lo + 512], xr, AF.Copy)

        # ---- mining on this core's 12 rows ----
        pmax = pers.tile([NB, 8], F32)
        pidx = pers.tile([NB, 8], U32)
        nmax = pers.tile([NB, 8], F32)
        nidx = pers.tile([NB, 8], U32)
        s12 = pers.tile([NB, M], F32, name="s12") if DEBUG else None
        mine_ctx = ExitStack()
        if F_RS:
            s12b = pers.tile([NB, M], BF16)
            nc.sync.dma_start(s12b, rs_out)
            nneg = pers.tile([NB, M], BF16)
            nc.vector.tensor_scalar_mul(nneg, s12b, -1.0)
            nc.vector.max(pmax, s12b)
            nc.vector.max_index(pidx, pmax, s12b)
            nc.vector.max(nmax, nneg)
            nc.vector.max_index(nidx, nmax, nneg)
            if DEBUG:
                nc.vector.tensor_copy(s12, s12b)
        else:
            # dual fold: +fsel and -fsel matmuls give the positive-mining
            # matrix and its negation straight in PSUM (no DVE negate)
            s8 = pers.tile([N, M], BF16, name="s8")
            for b in range(3):
                nc.sync.dma_start(
                    s8[:, b * 512:(b + 1) * 512],
                    rs_out[:, b * 512:(b + 1) * 512],
                )
            psumm = mine_ctx.enter_context(
                tc.tile_pool(name="psumm", bufs=1, space="PSUM"))
            pm = psumm.tile([NB, M], F32)
            nm = psumm.tile([NB, M], F32)
            for b in range(3):
                nc.tensor.matmul(
                    pm[:, b * 512:(b + 1) * 512], lhsT=fsel_sb,
                    rhs=s8[:, b * 512:(b + 1) * 512], start=True, stop=True,
                )
                nc.tensor.matmul(
                    nm[:, b * 512:(b + 1) * 512], lhsT=nfsel_sb,
                    rhs=s8[:, b * 512:(b + 1) * 512], start=True, stop=True,
                )
            nc.vector.max(pmax, pm)
            nc.vector.max_index(pidx, pmax, pm)
            nc.vector.max(nmax, nm)
            nc.vector.max_index(nidx, nmax, nm)
            if DEBUG:
                nc.vector.tensor_copy(s12, pm)

        # ---- phase 2: winner-row gathers + local distances ----
        y96p = pers.tile([N, DL], BF16)
        y96n = pers.tile([N, DL], BF16)
        if F_BIGGATHER == 1:
            nc.gpsimd.indirect_dma_start(
                out=y96p.rearrange("(b j) d -> b j d", j=S), out_offset=None,
                in_=lf4,
                in_offset=bass.IndirectOffsetOnAxis(ap=pidx[:, 0:1], axis=0),
            )
            nc.gpsimd.indirect_dma_start(
                out=y96n.rearrange("(b j) d -> b j d", j=S), out_offset=None,
                in_=lf4,
                in_offset=bass.IndirectOffsetOnAxis(ap=nidx[:, 0:1], axis=0),
            )
        elif F_BIGGATHER == 2:
            # replicate each winner index to its 8 stripe partitions via a
            # tiny f32 matmul (entries 8.0 -> 8*idx, exact for idx < 2^19),
            # add the per-partition stripe offset, one 96-row gather per set
            rep8_sb = ins["rep8_sb"]
            joff_sb = ins["joff_sb"]
            with tc.tile_pool(name="psumg", bufs=1, space="PSUM") as psumg:
                for iset, (idx, y96) in enumerate(((pidx, y96p), (nidx, y96n))):
                    idf = pers.tile([NB, 1], F32, tag="idf", bufs=2)
                    nc.vector.tensor_copy(idf, idx[:, 0:1])
                    rp = psumg.tile([N, 1], F32, tag="rp", bufs=2)
                    nc.tensor.matmul(rp, lhsT=rep8_sb, rhs=idf,
                                     start=True, stop=True)
                    pj96 = pers.tile([N, 1], U32, tag="pj96", bufs=2)
                    nc.vector.tensor_tensor(
                        out=pj96, in0=rp, in1=joff_sb, op=ALU.add
                    )
                    nc.gpsimd.indirect_dma_start(
                        out=y96, out_offset=None, in_=lf_v,
                        in_offset=bass.IndirectOffsetOnAxis(ap=pj96, axis=0),
                    )
        else:
            y3p = y96p.rearrange("(b j) d -> b j d", j=S)
            y3n = y96n.rearrange("(b j) d -> b j d", j=S)
            for iset, (idx, y3) in enumerate(((pidx, y3p), (nidx, y3n))):
                for j in range(S):
                    pj = pers.tile([NB, 1], U32, tag="pj", bufs=4)
                    nc.vector.tensor_scalar(
                        out=pj, in0=idx[:, :1], scalar1=8, scalar2=j,
                        op0=ALU.mult, op1=ALU.add,
                    )
                    nc.gpsimd.indirect_dma_start(
                        out=y3[:, j, :], out_offset=None, in_=lf_v,
                        in_offset=bass.IndirectOffsetOnAxis(ap=pj, axis=0),
                    )
        mine_ctx.close()

        d2t = pers.tile([N, 2 * S], F32)   # cols 0-7 pos, 8-15 neg
        with tc.tile_pool(name="workp2", bufs=3) as work2:
            # positive diffs first so ACT can chew squares continuously
            dtp = []
            for i in range(S):
                xsl = x_all[:, i * DL:(i + 1) * DL]
                dt = work2.tile([N, DL], BF16, tag=f"dtp{i}", bufs=1)
                nc.vector.tensor_tensor(out=dt, in0=y96p, in1=xsl,
                                        op=ALU.subtract)
                dtp.append(dt)
            for i in range(S):
                sqp = work2.tile([N, DL], BF16, tag="sqp")
                nc.scalar.activation(
                    sqp, dtp[i], AF.Square, accum_out=d2t[:, i:i + 1]
                )
            # negatives: subtracts split DVE/GpSimd, square-sums split ACT/DVE
            for i in range(S):
                xsl = x_all[:, i * DL:(i + 1) * DL]
                dtn = work2.tile([N, DL], BF16, tag="dtn")
                eng = nc.gpsimd if i < N_GPS_SUB else nc.vector
                eng.tensor_tensor(out=dtn, in0=y96n, in1=xsl, op=ALU.subtract)
                if i < S - N_DVE_NEG:
                    sqn = work2.tile([N, DL], BF16, tag="sqn")
                    nc.scalar.activation(
                        sqn, dtn, AF.Square, accum_out=d2t[:, S + i:S + i + 1]
                    )
                else:
                    sqn = work2.tile([N, DL], BF16, tag="sqn")
                    nc.vector.tensor_tensor(
                        out=sqn, in0=dtn, in1=dtn, op=ALU.mult
                    )
                    nc.vector.tensor_reduce(
                        d2t[:, S + i:S + i + 1], sqn,
                        axis=mybir.AxisListType.X, op=ALU.add,
                    )

            dcl = work2.tile([N, 2 * S], F32, tag="dcl", bufs=1)
            nc.vector.tensor_scalar_max(dcl, d2t, 1e-12)
            dsqr = work2.tile([N, 2 * S], F32, tag="dsqr", bufs=1)
            nc.scalar.activation(dsqr, dcl, AF.Sqrt)
            dsg = work2.tile([N, 2 * S], BF16, tag="dsg", bufs=1)
            nc.scalar.activation(dsg, dsqr, AF.Tanh, scale=0.5)
            # fold [96 (b,j), 16 (iset,i)] -> DP grid layout
            if F_SCAN:
                with tc.tile_pool(name="psf", bufs=1, space="PSUM") as psf_p:
                    psf = psf_p.tile([NB, 8 * 16], F32)
                    for j in range(S):
                        nc.tensor.matmul(
                            psf[:, j * 16:(j + 1) * 16],
                            lhsT=esel_sb[:, j * NB:(j + 1) * NB],
                            rhs=dsg, start=True, stop=True,
                        )
                    psf_v = psf.rearrange("p (j c) -> p c j", c=16)  # [12,16,8]
                    dg_v = dgrid.rearrange("p (i j) -> p i j", j=S)
                    nc.vector.tensor_copy(dg_v[0:NB], psf_v[:, 0:S, :])
                    nc.vector.tensor_copy(dg_v[32:32 + NB], psf_v[:, S:16, :])
            else:
                for iset in range(2):
                    with tc.tile_pool(name=f"psf{iset}", bufs=1,
                                      space="PSUM") as psf_p:
                        psf = psf_p.tile([NB, 64], F32)
                        for j in range(S):
                            nc.tensor.matmul(
                                psf[:, j * S:(j + 1) * S],
                                lhsT=esel_sb[:, j * NB:(j + 1) * NB],
                                rhs=dsg[:, iset * S:(iset + 1) * S],
                                start=True, stop=True,
                            )
                        dst = dD[iset * 32:iset * 32 + NB, 0:72].rearrange(
                            "p (i j) -> p j i", j=9
                        )[:, 1:9, :]
                        nc.vector.tensor_copy(
                            dst, psf.rearrange("p (j i) -> p j i", i=S)
                        )

        # ---- shortest-path DP ----
        if F_SCAN:
            for i in range(S):
                prev = bigrow if i == 0 else dp_sb[:, (i - 1) * S:i * S]
                nc.vector.tensor_tensor_scan(
                    out=dp_sb[:, i * S:(i + 1) * S],
                    data0=prev,
                    data1=dgrid[:, i * S:(i + 1) * S],
                    initial=0.0 if i == 0 else BIGDP,
                    op0=ALU.min, op1=ALU.add,
                )
            dp_last = dp_sb[:, 63:64]
        else:
            for kdiag in range(2, 17):
                lo = max(1, kdiag - 8)
                hi = min(8, kdiag - 1)
                cnt = hi - lo + 1
                f0 = 9 * lo + (kdiag - lo)
                t = pers.tile([64, 8], F32, tag="dptmp", bufs=2)
                nc.vector.tensor_tensor(
                    out=t[:, :cnt],
                    in0=dp[:, f0 - 9:f0 - 9 + 8 * (cnt - 1) + 1:8],
                    in1=dp[:, f0 - 1:f0 - 1 + 8 * (cnt - 1) + 1:8],
                    op=ALU.min,
                )
                nc.vector.tensor_tensor(
                    out=dp[:, f0:f0 + 8 * (cnt - 1) + 1:8],
                    in0=t[:, :cnt],
                    in1=dD[:, f0 - 9:f0 - 9 + 8 * (cnt - 1) + 1:8],
                    op=ALU.add,
                )
            dp_last = dp[:, 80:81]

        # ---- loss: relu(ap - an + margin), partial sum over 12 centers ----
        with tc.tile_pool(name="psum3", bufs=1, space="PSUM") as psum3:
            dps = psum3.tile([NB, 1], F32)
            nc.tensor.matmul(dps, lhsT=wsel_sb, rhs=dp_last, start=True, stop=True)
            r12 = pers.tile([NB, 1], F32)
            nc.vector.tensor_scalar(
                out=r12, in0=dps, scalar1=marg, scalar2=0.0,
                op0=ALU.add, op1=ALU.max,
            )
            lsum = psum3.tile([1, 1], F32)
            nc.tensor.matmul(lsum, lhsT=r12, rhs=ones_c, start=True, stop=True)
            out_sb = pers.tile([1, 1], F32)
            nc.vector.tensor_copy(out_sb, lsum)
        nc.sync.dma_start(outs["out"], out_sb)

        if DEBUG:
            nc.sync.dma_start(outs["dbg_pidx"], pidx)
            nc.sync.dma_start(outs["dbg_nidx"], nidx)
            nc.sync.dma_start(outs["dbg_d2t"], d2t)
            if F_SCAN:
                nc.sync.dma_start(outs["dbg_dgrid"], dgrid)
                nc.sync.dma_start(outs["dbg_dp"], dp_sb)
            nc.sync.dma_start(outs["dbg_s12"], s12)


def build_program():
    nc = bacc.Bacc(
        "TRN2", target_bir_lowering=False, debug=False,
        enable_asserts=False, num_devices=NCORES,
    )
    ins = {
        "fnat": nc.dram_tensor("fnat", [M, DPC], BF16, kind="ExternalInput").ap(),
        "ftT": nc.dram_tensor("ftT", [DPC, M], BF16, kind="ExternalInput").ap(),
        "lf": nc.dram_tensor("lf", [M, D], BF16, kind="ExternalInput").ap(),
        "um": nc.dram_tensor("um", [33, N], BF16, kind="ExternalInput").ap(),
        "vm": nc.dram_tensor("vm", [33, M], BF16, kind="ExternalInput").ap(),
        "wsel": nc.dram_tensor("wsel", [64, NB], F32, kind="ExternalInput").ap(),
        "ident": nc.dram_tensor("ident", [128, 128], F32, kind="ExternalInput").ap(),
        "esel": nc.dram_tensor("esel", [N, N], BF16, kind="ExternalInput").ap(),
        "aavg": nc.dram_tensor("aavg", [128, NT * N], BF16, kind="ExternalInput").ap(),
        "fsel": nc.dram_tensor("fsel", [N, NB], BF16, kind="ExternalInput").ap(),
        "repsel": nc.dram_tensor("repsel", [NB, N], BF16, kind="ExternalInput").ap(),
        "rep8": nc.dram_tensor("rep8", [NB, N], F32, kind="ExternalInput").ap(),
        "joff": nc.dram_tensor("joff", [N, 1], F32, kind="ExternalInput").ap(),
        "nfsel": nc.dram_tensor("nfsel", [N, NB], BF16, kind="ExternalInput").ap(),
    }
    outs = {"out": nc.dram_tensor("out", [1, 1], F32, kind="ExternalOutput").ap()}
    if DEBUG:
        outs["dbg_pidx"] = nc.dram_tensor(
            "dbg_pidx", [NB, 8], U32, kind="ExternalOutput").ap()
        outs["dbg_nidx"] = nc.dram_tensor(
            "dbg_nidx", [NB, 8], U32, kind="ExternalOutput").ap()
        outs["dbg_d2t"] = nc.dram_tensor(
            "dbg_d2t", [N, 2 * S], F32, kind="ExternalOutput").ap()
        if F_SCAN:
            outs["dbg_dgrid"] = nc.dram_tensor(
                "dbg_dgrid", [64, 64], F32, kind="ExternalOutput").ap()
            outs["dbg_dp"] = nc.dram_tensor(
                "dbg_dp", [64, 64], F32, kind="ExternalOutput").ap()
        outs["dbg_s12"] = nc.dram_tensor(
            "dbg_s12", [NB, M], F32, kind="ExternalOutput").ap()
    with tile.TileContext(nc) as tc:
        build_body(tc, outs, ins)
    nc.compile()
    return nc


def make_in_maps(feats, labels, local_features):
    bf16 = mybir.dt.np(BF16)
    feats = np.asarray(feats, dtype=np.float32).astype(bf16)
    labels = np.asarray(labels).astype(np.int64)
    lf_flat = np.ascontiguousarray(
        np.asarray(local_features, dtype=np.float32)
        .transpose(0, 2, 1).reshape(M, D)
    ).astype(bf16)
    anchors = labels[::K]  # [96]
    # mask factors: sum_r um[r,n]*vm[r,m] = -BIGM * (anchors[n] != labels[m])
    um = np.zeros((33, N), dtype=np.float32)
    vm = np.zeros((33, M), dtype=np.float32)
    um[0, :] = -BIGM
    vm[0, :] = 1.0
    for ell in range(L):
        um[1 + ell, :] = BIGM * (anchors == ell)
        vm[1 + ell, :] = (labels == ell)
    um = um.astype(bf16)
    vm = vm.astype(bf16)
    # fold selector: exchange-out row (NB*i + b) -> out row b
    fsel = np.zeros((N, NB), dtype=np.float32)
    for i in range(NCORES):
        for b in range(NB):
            fsel[NB * i + b, b] = 1.0
    fsel_b = fsel.astype(bf16)
    nfsel_b = (-fsel).astype(bf16)
    # replicate selector: center b -> partitions b*8..b*8+8 (fallback)
    repsel = np.zeros((NB, N), dtype=np.float32)
    for b in range(NB):
        repsel[b, b * S:(b + 1) * S] = 1.0
    repsel = repsel.astype(bf16)
    wsel = np.zeros((64, NB), dtype=np.float32)
    wsel[0:NB, :] = np.eye(NB)
    wsel[32:32 + NB, :] = -np.eye(NB)
    ident = np.eye(128, dtype=np.float32)
    esel = np.zeros((N, N), dtype=np.float32)
    for b in range(NB):
        for j in range(S):
            esel[b * S + j, j * NB + b] = 1.0
    esel = esel.astype(bf16)
    # packed chunk-averaging matrix: [128, t*96+n]
    aavg = np.zeros((M, N), dtype=np.float32)
    aavg[np.arange(M), np.arange(M) // K] = 1.0 / K
    aavg_p = np.ascontiguousarray(
        aavg.reshape(NT, 128, N).transpose(1, 0, 2).reshape(128, NT * N)
    ).astype(bf16)
    in_maps = []
    for c in range(NCORES):
        sl = feats[:, c * DPC:(c + 1) * DPC]
        in_maps.append({
            "fnat": np.ascontiguousarray(sl),
            "ftT": np.ascontiguousarray(sl.T),
            "lf": lf_flat,
            "um": um,
            "vm": vm,
            "wsel": wsel,
            "ident": ident,
            "esel": esel,
            "aavg": aavg_p,
            "fsel": fsel_b,
            "repsel": repsel,
            "rep8": 8.0 * repsel.astype(np.float32),
            "joff": np.tile(np.arange(S, dtype=np.float32), NB)[:, None],
            "nfsel": nfsel_b,
        })
    return in_maps


_NC_CACHE = None


def _get_nc():
    global _NC_CACHE
    if _NC_CACHE is None:
        _NC_CACHE = build_program()
    return _NC_CACHE


def run(feats, labels, local_features, trace=False, **kwargs):
    nc = _get_nc()
    in_maps = make_in_maps(feats, labels, local_features)
    res = bass_utils.run_bass_kernel_spmd(
        nc, in_maps, core_ids=list(range(NCORES)), trace=trace, **kwargs
    )
    partial = sum(float(r["out"][0, 0]) for r in res.results)
    return np.float32(partial / N), res


def kernel(feats, labels, local_features):
    loss, _ = run(feats, labels, local_features)
    return loss
